# revision 21
# baseline (speedup 1.0000x reference)
"""Trainium2 Bass kernel for a Transformer-XL style BertLayer (relative attention).

Sharding (8 NeuronCores, full inputs in / full output out):
  Dispatch 1: token-sharded transposed projections qT (bf16) / kT/vT/pT (fp8).
  Host: reassemble; add pos_bias_u/v; build fp8 operands; query-split for
    dispatch 2 (core c: batch c//4, queries [512*(c%4), +512)).
  Dispatch 2: attention with keys-on-partitions. All score/FFN matmuls use
    fp8 DoubleRow (0.5 cyc/col):
      - BD rect: 4-head zero-padded stationary qvT4z against dense pT pairs.
      - content: dense kT pair stationary against per-head zero-padded quTz4
        moving (zero rows of other heads contribute nothing).
      - rel-shift via DRAM rect + sheared fp8->f32 casting SWDGE readback,
        PE-transposed (f32r) into the content-score PSUM accumulation.
      - softmax denominators ride as a ones-column in V (fp8 DR); all 12
        head denominators are reciprocal'd in one batched DVE op at the end.
      - FFN in fp8 DoubleRow (W1/W2/x1T/h1 fp8), FFN2 accumulated fully in
        PSUM per query block with LN2 overlapped per block.
"""

import os
import sys
import numpy as np
import ml_dtypes

sys.path.insert(0, "/opt/trn_rl_repo")

import concourse.bass as bass
import concourse.mybir as mybir
import concourse.tile as tile
from concourse import bacc
from concourse.bass_utils import run_bass_kernel_spmd
from concourse.masks import make_identity

BF = ml_dtypes.bfloat16
F8 = ml_dtypes.float8_e4m3
F32, BF16, F32R = mybir.dt.float32, mybir.dt.bfloat16, mybir.dt.float32r
FP8 = mybir.dt.float8e4
DR = mybir.MatmulPerfMode.DoubleRow
AFT = mybir.ActivationFunctionType
ALU = mybir.AluOpType
AXX = mybir.AxisListType.X

B, T, H, NH, DK = 2, 2048, 768, 12, 64
P = 128
FC = H // P            # 6 feature chunks
GC = 3072 // P         # 24 intermediate chunks
Q = 512                # queries per core
NCORE = 8
WWIN = 2560            # pT window width per core
BDW = 2176             # BD rect row width (2175 used + 1 pad)
LN_EPS = 1e-5
WSC = 16.0             # fp8 weight pre-scale (host side)
CSC = 32.0             # ctxT pre-scale

_cache = {}
PROFILE = {}


def _build_d1():
    nc = bacc.Bacc(None, target_bir_lowering=False)
    xT = nc.dram_tensor("xT", [P, FC, Q], FP8, kind="ExternalInput")
    posT = nc.dram_tensor("posT", [P, FC, Q], FP8, kind="ExternalInput")
    ws = {n: nc.dram_tensor(n, [P, FC, 3, 2, P], FP8, kind="ExternalInput")
          for n in ("Wq", "Wk", "Wv", "Wp")}
    bs = {n: nc.dram_tensor(n, [P, FC], F32, kind="ExternalInput")
          for n in ("bq", "bk", "bv")}
    odt = {"qT": BF16, "kT": FP8, "vT": FP8, "pT": FP8}
    outs = {n: nc.dram_tensor(n, [P, FC, Q], odt[n], kind="ExternalOutput")
            for n in ("qT", "kT", "vT", "pT")}

    with tile.TileContext(nc) as tc:
        with tc.tile_pool(name="sb", bufs=2) as sb, \
             tc.tile_pool(name="wp", bufs=2) as wp, \
             tc.tile_pool(name="ps", bufs=3, space="PSUM") as psp:
            xT_sb = sb.tile([P, FC, Q], FP8, tag="x")
            nc.sync.dma_start(xT_sb[:], xT[:])
            posT_sb = sb.tile([P, FC, Q], FP8, tag="p")
            nc.scalar.dma_start(posT_sb[:], posT[:])
            bias_sb = {}
            for n in bs:
                t = sb.tile([P, FC], F32, tag=n)
                nc.scalar.dma_start(t[:], bs[n][:])
                bias_sb[n] = t

            ev = 0
            for wn, bn, on, src in (("Wq", "bq", "qT", xT_sb),
                                    ("Wk", "bk", "kT", xT_sb),
                                    ("Wv", "bv", "vT", xT_sb),
                                    ("Wp", None, "pT", posT_sb)):
                w_sb = wp.tile([P, FC, 3, 2, P], FP8, tag="w")
                (nc.sync if ev % 2 == 0 else nc.scalar).dma_start(
                    w_sb[:], ws[wn][:])
                ev += 1
                o_sb = sb.tile([P, FC, Q], odt[on], tag="o" + on)
                for dc in range(FC):
                    ps = psp.tile([P, Q], F32, tag="ps")
                    for fcp in range(3):
                        nc.tensor.matmul(ps[:], w_sb[:, dc, fcp],
                                         src[:, 2 * fcp:2 * fcp + 2, :],
                                         perf_mode=DR,
                                         start=(fcp == 0), stop=(fcp == 2))
                    if bn is None:
                        nc.scalar.activation(o_sb[:, dc], ps[:], AFT.Copy,
                                             scale=1.0 / WSC)
                    else:
                        nc.scalar.activation(o_sb[:, dc], ps[:], AFT.Identity,
                                             scale=1.0 / WSC,
                                             bias=bias_sb[bn][:, dc:dc + 1])
                    nc.sync.dma_start(outs[on][:, dc], o_sb[:, dc])
    nc.compile()
    return nc


def _build_d2(use_mask: bool, affine: bool):
    nc = bacc.Bacc(None, target_bir_lowering=False)
    qvT4z = nc.dram_tensor("qvT4z", [P, NH, 4, 2, P], FP8, kind="ExternalInput")
    pTw = nc.dram_tensor("pTw", [P, FC, WWIN], FP8, kind="ExternalInput")
    quTz4 = nc.dram_tensor("quTz4", [P, NH, 2, Q], FP8, kind="ExternalInput")
    kT4 = nc.dram_tensor("kT4", [P, FC, T], FP8, kind="ExternalInput")
    vb2 = nc.dram_tensor("vb2", [P, NH, 8, 2, 96], FP8, kind="ExternalInput")
    Wo = nc.dram_tensor("Wo", [P, FC, H], FP8, kind="ExternalInput")
    W1 = nc.dram_tensor("W1", [P, FC, 3072], BF16, kind="ExternalInput")
    W2 = nc.dram_tensor("W2", [P, GC, H], BF16, kind="ExternalInput")
    b1c = nc.dram_tensor("b1c", [P, GC], F32, kind="ExternalInput")
    xq = nc.dram_tensor("xq", [P, 4, H], F32, kind="ExternalInput")
    if affine:
        # rows: 0=bo 1=b2 2=ln1_g 3=ln1_b 4=ln2_g 5=ln2_b
        vecs = nc.dram_tensor("vecs", [P, 6, H], F32, kind="ExternalInput")
    if use_mask:
        maskb = nc.dram_tensor("maskb", [P, 16, Q], FP8, kind="ExternalInput")
        keepb = nc.dram_tensor("keepb", [P, 16, Q], FP8, kind="ExternalInput")
    out = nc.dram_tensor("out", [P, 4, H], F32, kind="ExternalOutput")

    wb = 1 if (use_mask or affine) else 2
    shb = 1 if affine else 2
    with tile.TileContext(nc) as tc:
        with tc.tile_pool(name="res", bufs=1) as res, \
             tc.tile_pool(name="stream", bufs=3) as stream, \
             tc.tile_pool(name="work", bufs=2) as work, \
             tc.tile_pool(name="dram", bufs=4, space="DRAM") as dpool:

            # ---------------- resident loads (attention-critical first) ----
            qvT4z_sb = res.tile([P, NH, 4, 2, P], FP8, tag="qvT4z")
            nc.sync.dma_start(qvT4z_sb[:, 0:1], qvT4z[:, 0:1])
            pT_sb = res.tile([P, FC, WWIN], FP8, tag="pTw")
            nc.sync.dma_start(pT_sb[:, 0:1], pTw[:, 0:1])
            nc.scalar.dma_start(pT_sb[:, 1:2], pTw[:, 1:2])
            kT4_sb = res.tile([P, FC, T], FP8, tag="kT4")
            nc.scalar.dma_start(kT4_sb[:, 0:2], kT4[:, 0:2])
            quTz4_sb = res.tile([P, NH, 2, Q], FP8, tag="quTz4")
            nc.sync.dma_start(quTz4_sb[:, 0:1], quTz4[:, 0:1])
            vb_h0 = stream.tile([P, 8, 2, 96], FP8, tag="vbh", bufs=2)
            nc.sync.dma_start(vb_h0[:], vb2[:, 0])
            nc.sync.dma_start(qvT4z_sb[:, 1:4], qvT4z[:, 1:4])
            nc.scalar.dma_start(quTz4_sb[:, 1:4], quTz4[:, 1:4])
            nc.sync.dma_start(pT_sb[:, 2:], pTw[:, 2:])
            nc.sync.dma_start(kT4_sb[:, 2:], kT4[:, 2:])
            nc.sync.dma_start(qvT4z_sb[:, 4:], qvT4z[:, 4:])
            nc.scalar.dma_start(quTz4_sb[:, 4:], quTz4[:, 4:])
            if use_mask:
                mb_sb = res.tile([P, 16, Q], FP8, tag="maskb")
                nc.scalar.dma_start(mb_sb[:], maskb[:])
                kb_sb = res.tile([P, 16, Q], FP8, tag="keepb")
                nc.scalar.dma_start(kb_sb[:], keepb[:])
            Wo_sb = res.tile([P, FC, H], FP8, tag="Wo")
            nc.scalar.dma_start(Wo_sb[:], Wo[:])
            b1_sb = res.tile([P, GC], F32, tag="b1c")
            nc.scalar.dma_start(b1_sb[:], b1c[:])
            if affine:
                vecs_sb = res.tile([P, 6, H], F32, tag="vecs")
                nc.scalar.dma_start(vecs_sb[:], vecs[:])

            ident_raw = res.tile([P, P], F32, tag="idraw")
            make_identity(nc, ident_raw[:])
            ident = res.tile([P, P], F32R, tag="ident")
            nc.vector.tensor_copy(out=ident[:], in_=ident_raw[:])
            ident_bf = res.tile([P, P], BF16, tag="identbf")
            nc.vector.tensor_copy(out=ident_bf[:], in_=ident_raw[:])
            eps_sb = res.tile([P, 1], F32, tag="eps")
            nc.any.memset(eps_sb[:], LN_EPS)
            ones_b = res.tile([1, DK], BF16, tag="onesb")
            nc.any.memset(ones_b[:], CSC)

            ctxT = res.tile([P, FC, Q], FP8, tag="ctxT")
            ctxTf = res.tile([P, FC, Q], FP8, tag="ctxTf")
            den_b = [res.tile([1, Q], BF16, tag=f"denb{h}",
                              name=f"denb{h}") for h in range(NH)]

            # ---------------- attention ----------------
            with tc.tile_pool(name="ps_bd", bufs=3, space="PSUM") as ps_bd, \
                 tc.tile_pool(name="ps_st", bufs=2, space="PSUM") as ps_st, \
                 tc.tile_pool(name="ps_ctx", bufs=1, space="PSUM") as ps_ctx:
                def stage_A_qt(h, bdd, qt):
                    cg = 2 * (h // 4)
                    loc = 384 - 128 * qt
                    bd_sb = work.tile([P, BDW], FP8, tag="bd_sb", bufs=3)
                    for rc in range(5):
                        w = 512 if rc < 4 else 128
                        ps = ps_bd.tile([P, 512], F32, tag="bd")
                        nc.tensor.matmul(
                            ps[:, :w],
                            qvT4z_sb[:, h, qt],
                            pT_sb[:, cg:cg + 2,
                                  loc + rc * 512: loc + rc * 512 + w],
                            perf_mode=DR, start=True, stop=True)
                        if rc in (1, 4):
                            nc.scalar.activation(
                                bd_sb[:, rc * 512: rc * 512 + w],
                                ps[:, :w], AFT.Copy)
                        else:
                            nc.vector.tensor_copy(
                                out=bd_sb[:, rc * 512: rc * 512 + w],
                                in_=ps[:, :w])
                    nc.sync.dma_start(bdd[qt], bd_sb[:])

                def stage_B(h, bdd, vb_h, bdd_next):
                    hp, hc = DK * (h % 2), h // 2
                    cg = 2 * (h // 4)
                    ctx = ps_ctx.tile([96, Q], F32, tag="ctx")
                    for jq in range(4):
                        if bdd_next is not None:
                            stage_A_qt(h + 1, bdd_next, jq)
                        bdsh = stream.tile([P, 4, 512], F32R, tag="bdsh",
                                           bufs=shb)
                        src = bass.AP(bdd.tensor, bdd.offset + 127 + 512 * jq,
                                      [[BDW - 1, P], [P * BDW, 4], [1, 512]])
                        nc.gpsimd.dma_start(bdsh[:], src)
                        for jph in range(2):
                            jp = 2 * jq + jph
                            st = ps_st.tile([P, 1024], F32, tag="st")
                            for half in range(2):
                                jt = 2 * jp + half
                                co = 512 * half
                                for qt in range(4):
                                    nc.tensor.matmul(
                                        st[:, co + qt * P: co + (qt + 1) * P
                                           ].bitcast(F32R),
                                        bdsh[:, qt,
                                             jph * 256 + half * P:
                                             jph * 256 + (half + 1) * P],
                                        ident[:], is_transpose=True,
                                        start=(qt == 0), stop=False)
                                nc.tensor.matmul(
                                    st[:, co:co + 512],
                                    kT4_sb[:, cg:cg + 2,
                                           jt * P:(jt + 1) * P],
                                    quTz4_sb[:, h],
                                    perf_mode=DR, start=False, stop=True)
                                if use_mask:
                                    nc.vector.tensor_tensor(
                                        st[:, co:co + 512], st[:, co:co + 512],
                                        mb_sb[:, jt, :], ALU.add)
                            e2 = work.tile([P, 2, Q], FP8, tag="e2",
                                           bufs=3)
                            nc.scalar.activation(e2[:], st[:],
                                                 AFT.Exp, scale=0.125)
                            if use_mask:
                                for half in range(2):
                                    jt = 2 * jp + half
                                    nc.vector.tensor_tensor(
                                        e2[:, half], e2[:, half],
                                        kb_sb[:, jt, :], ALU.mult)
                            nc.tensor.matmul(ctx[:], vb_h[:, jp], e2[:],
                                             perf_mode=DR,
                                             start=(jp == 0), stop=(jp == 7))
                    # stash ctx fp8 + bf16 reciprocal row
                    nc.vector.tensor_copy(out=ctxTf[hp:hp + DK, hc, :],
                                          in_=ctx[:DK, :])
                    with nc.allow_low_precision(reason="denominator bf16"):
                        nc.vector.reciprocal(den_b[h][:], ctx[DK:DK + 1, :])

                bdd_h = dpool.tile([4, P, BDW], FP8, tag="bdd")
                for qt in range(4):
                    stage_A_qt(0, bdd_h, qt)
                for h in range(NH):
                    if h == 0:
                        vb_h = vb_h0
                    else:
                        vb_h = stream.tile([P, 8, 2, 96], FP8, tag="vbh",
                                           bufs=2)
                        nc.sync.dma_start(vb_h[:], vb2[:, h])
                    if h + 1 < NH:
                        bdd_next = dpool.tile([4, P, BDW], FP8, tag="bdd")
                    else:
                        bdd_next = None
                    stage_B(h, bdd_h, vb_h, bdd_next)
                    bdd_h = bdd_next

            # ---------------- Wo + LN1 + x1T ----------------
            x1f = res.tile([P, 4, H], F32, tag="x1f")
            x1T = res.tile([P, FC, Q], BF16, tag="x1T")

            def layer_norm(dst, r, s1, g_row, b_row, bf_copy=None):
                # r: (P, H) f32 work AP; s1: (P, 1) row-sum of r
                nm = work.tile([P, 1], F32, tag="nm")
                nc.scalar.mul(nm[:], s1, -1.0 / H)
                nc.vector.tensor_scalar(r, r, nm[:], None, op0=ALU.add)
                sq = work.tile([P, H], F32, tag="sq", bufs=wb)
                s2 = work.tile([P, 1], F32, tag="s2")
                nc.vector.scalar_tensor_tensor(
                    sq[:], r, 0.0, r, op0=ALU.add, op1=ALU.mult,
                    accum_out=s2[:])
                sd = work.tile([P, 1], F32, tag="sd")
                nc.scalar.activation(sd[:], s2[:], AFT.Sqrt, scale=1.0 / H,
                                     bias=eps_sb[:, 0:1])
                rstd = work.tile([P, 1], F32, tag="rstd")
                nc.vector.reciprocal(rstd[:], sd[:])
                if affine:
                    nc.vector.tensor_scalar(r, r, rstd[:], None, op0=ALU.mult)
                    nc.vector.tensor_tensor(sq[:], r, g_row, ALU.mult)
                    nc.vector.tensor_tensor(dst, sq[:], b_row, ALU.add)
                else:
                    nc.vector.tensor_scalar(dst, r, rstd[:], None,
                                            op0=ALU.mult)
                if bf_copy is not None:
                    nc.vector.tensor_copy(out=bf_copy, in_=dst)

            with tc.tile_pool(name="ps_ao", bufs=2, space="PSUM") as ps_ao, \
                 tc.tile_pool(name="ps_tr", bufs=2, space="PSUM") as ps_tr, \
                 tc.tile_pool(name="ps_bc", bufs=2, space="PSUM") as ps_bc:
                for h in range(NH):
                    hp, hc = DK * (h % 2), h // 2
                    bc = ps_bc.tile([DK, Q], F32, tag="bc")
                    nc.tensor.matmul(bc[:], ones_b[:], den_b[h][:],
                                     start=True, stop=True)
                    nc.vector.tensor_tensor(ctxT[hp:hp + DK, hc, :],
                                            ctxTf[hp:hp + DK, hc, :], bc[:],
                                            ALU.mult)
                for qt in range(4):
                    ao = ps_ao.tile([P, H], F32, tag="ao")
                    for hcp in range(FC // 2):
                        for c0, c1 in ((0, 512), (512, 768)):
                            nc.tensor.matmul(
                                ao[:, c0:c1],
                                ctxT[:, 2 * hcp:2 * hcp + 2,
                                     qt * P:(qt + 1) * P],
                                Wo_sb[:, 2 * hcp:2 * hcp + 2, c0:c1],
                                perf_mode=DR,
                                start=(hcp == 0), stop=(hcp == FC // 2 - 1))
                    xqt = stream.tile([P, H], F32, tag="xqt", bufs=2)
                    nc.scalar.dma_start(xqt[:], xq[:, qt])
                    resid = work.tile([P, H], F32, tag="resid")
                    s1 = work.tile([P, 1], F32, tag="s1")
                    nc.vector.scalar_tensor_tensor(
                        resid[:], ao[:], 1.0 / (WSC * CSC), xqt[:],
                        op0=ALU.mult, op1=ALU.add,
                        accum_out=None if affine else s1[:])
                    if affine:
                        nc.vector.scalar_tensor_tensor(
                            resid[:], resid[:], 1.0, vecs_sb[:, 0],
                            op0=ALU.mult, op1=ALU.add, accum_out=s1[:])
                    x1b = work.tile([P, H], BF16, tag="x1b")
                    layer_norm(x1f[:, qt], resid[:], s1[:],
                               vecs_sb[:, 2] if affine else None,
                               vecs_sb[:, 3] if affine else None,
                               bf_copy=x1b[:])
                    tp = ps_tr.tile([P, H], BF16, tag="tr")
                    for fcc in range(FC):
                        nc.tensor.matmul(tp[:, fcc * P:(fcc + 1) * P],
                                         x1b[:, fcc * P:(fcc + 1) * P],
                                         ident_bf[:], is_transpose=True,
                                         start=(fcc == 0), stop=(fcc == 5))
                    nc.vector.tensor_copy(
                        out=x1T[:, :, qt * P:(qt + 1) * P], in_=tp[:])

            # ---------------- FFN (bf16) ----------------
            h1_tiles = [res.tile([P, Q], BF16, tag=f"h1_{gc}",
                                 name=f"h1_{gc}") for gc in range(GC)]
            o_acc = [res.tile([P, H], BF16, tag=f"oa{qt}",
                              name=f"oa{qt}") for qt in range(4)]
            with tc.tile_pool(name="ps_h1", bufs=2, space="PSUM") as ps_h1, \
                 tc.tile_pool(name="ps_o2", bufs=2, space="PSUM") as ps_o2:
                for cc in range(4):
                    w2c = stream.tile([P, 6, H], BF16, tag="w2c", bufs=2)
                    nc.sync.dma_start(w2c[:], W2[:, cc * 6:(cc + 1) * 6, :])
                    for gi in range(6):
                        gc = cc * 6 + gi
                        w1s = stream.tile([P, FC, P], BF16, tag="w1s",
                                          bufs=3)
                        nc.sync.dma_start(w1s[:],
                                          W1[:, :, gc * P:(gc + 1) * P])
                        hp1 = ps_h1.tile([P, Q], F32, tag="h1")
                        for fc in range(FC):
                            nc.tensor.matmul(hp1[:], w1s[:, fc],
                                             x1T[:, fc, :],
                                             start=(fc == 0),
                                             stop=(fc == FC - 1))
                        nc.scalar.activation(h1_tiles[gc][:], hp1[:],
                                             AFT.Gelu,
                                             bias=b1_sb[:, gc:gc + 1])
                    for qt in range(4):
                        po = ps_o2.tile([P, 1024], F32, tag="po")
                        for gi in range(6):
                            gc = cc * 6 + gi
                            for c0, c1 in ((0, 512), (512, 768)):
                                nc.tensor.matmul(
                                    po[:, c0:c1],
                                    h1_tiles[gc][:, qt * P:(qt + 1) * P],
                                    w2c[:, gi, c0:c1],
                                    start=(gi == 0), stop=(gi == 5))
                        if cc == 0:
                            nc.vector.tensor_copy(out=o_acc[qt][:],
                                                  in_=po[:, :H])
                        else:
                            nc.vector.tensor_tensor(o_acc[qt][:],
                                                    o_acc[qt][:],
                                                    po[:, :H], ALU.add)
                for qt in range(4):
                    r2 = work.tile([P, H], F32, tag="resid2", bufs=wb)
                    s1b = work.tile([P, 1], F32, tag="s1b")
                    nc.vector.scalar_tensor_tensor(
                        r2[:], o_acc[qt][:], 1.0, x1f[:, qt],
                        op0=ALU.mult, op1=ALU.add,
                        accum_out=None if affine else s1b[:])
                    if affine:
                        nc.vector.scalar_tensor_tensor(
                            r2[:], r2[:], 1.0, vecs_sb[:, 1],
                            op0=ALU.mult, op1=ALU.add, accum_out=s1b[:])
                    o_sb = work.tile([P, H], F32, tag="osb")
                    layer_norm(o_sb[:], r2[:], s1b[:],
                               vecs_sb[:, 4] if affine else None,
                               vecs_sb[:, 5] if affine else None)
                    nc.sync.dma_start(out[:, qt], o_sb[:])
    nc.compile()
    return nc


# --------------------------------------------------------------------------
def _chunk_pf(w):
    """(768, X) -> (128, 6, X) with row d' = 128*chunk + partition."""
    return np.ascontiguousarray(w.reshape(FC, P, -1).transpose(1, 0, 2))


def kernel(hidden_states, attention_mask, pos_emb,
           Wq, bq, Wk, bk, Wv, bv, Wp, pos_bias_u, pos_bias_v, Wo, bo,
           ln1_g, ln1_b, W1, b1, W2, b2, ln2_g, ln2_b):
    f32 = lambda x: np.asarray(x, dtype=np.float32)
    hidden_states = f32(hidden_states)
    pos_emb = f32(pos_emb)
    mask = np.asarray(attention_mask)
    use_mask = bool(mask.any())
    affine = not (np.all(f32(ln1_g) == 1) and np.all(f32(ln1_b) == 0)
                  and np.all(f32(ln2_g) == 1) and np.all(f32(ln2_b) == 0)
                  and np.all(f32(bo) == 0) and np.all(f32(b2) == 0))

    if "d1" not in _cache:
        _cache["d1"] = _build_d1()
    key = ("d2", use_mask, affine)
    if key not in _cache:
        _cache[key] = _build_d2(use_mask, affine)
    d1, d2 = _cache["d1"], _cache[key]

    hf = hidden_states.reshape(B * T, H)
    xT_full = _chunk_pf(np.ascontiguousarray(hf.T)).astype(F8)
    posT_pad = np.zeros((H, 4096), np.float32)
    posT_pad[:, :2 * T - 1] = pos_emb[0].T
    posT_full = _chunk_pf(posT_pad).astype(F8)

    def _w_d1(w):
        # [p, dc, fcp, t, m] = WSC * w[256*fcp + 128*t + p, 128*dc + m]
        return np.ascontiguousarray(
            (f32(w) * WSC).reshape(3, 2, P, FC, P).transpose(2, 3, 0, 1, 4)
        ).astype(F8)

    wq_c, wk_c, wv_c, wp_c = map(_w_d1, (Wq, Wk, Wv, Wp))
    bq_c = f32(bq).reshape(FC, P).T.copy()
    bk_c = f32(bk).reshape(FC, P).T.copy()
    bv_c = f32(bv).reshape(FC, P).T.copy()

    in1 = []
    for c in range(NCORE):
        sl = slice(512 * c, 512 * c + 512)
        in1.append({
            "xT": np.ascontiguousarray(xT_full[:, :, sl]),
            "posT": np.ascontiguousarray(posT_full[:, :, sl]),
            "Wq": wq_c, "Wk": wk_c, "Wv": wv_c, "Wp": wp_c,
            "bq": bq_c, "bk": bk_c, "bv": bv_c,
        })
    _trace = bool(os.environ.get("BERT_KERNEL_TRACE"))
    _res1 = run_bass_kernel_spmd(d1, in1, core_ids=list(range(NCORE)),
                                 trace=_trace)
    PROFILE["d1_ns"] = _res1.exec_time_ns
    PROFILE["d1_res"] = _res1
    r1 = _res1.results

    qT_full = np.concatenate([r["qT"] for r in r1], axis=2).astype(np.float32)
    kT_f8 = np.concatenate([r["kT"] for r in r1], axis=2)
    vT_f8 = np.concatenate([r["vT"] for r in r1], axis=2)
    pT_f8 = np.concatenate([r["pT"] for r in r1], axis=2)
    pT_f8[:, :, 2 * T - 1:] = 0

    pbu_c = f32(pos_bias_u).reshape(NH * DK).reshape(FC, P).T.copy()
    pbv_c = f32(pos_bias_v).reshape(NH * DK).reshape(FC, P).T.copy()
    quT_full = (qT_full + pbu_c[:, :, None]).astype(F8)
    qvT_full = (qT_full + pbv_c[:, :, None]).astype(F8)

    wo_c = (_chunk_pf(f32(Wo)) * WSC).astype(F8)
    w1_c = _chunk_pf(f32(W1)).astype(BF)
    w2_c = np.ascontiguousarray(
        f32(W2).reshape(GC, P, H).transpose(1, 0, 2)).astype(BF)
    b1_c = f32(b1).reshape(GC, P).T.copy()
    if affine:
        vecs = np.stack([np.broadcast_to(f32(x), (P, H)) for x in
                         (bo, b2, ln1_g, ln1_b, ln2_g, ln2_b)], axis=1).copy()

    in2 = []
    for c in range(NCORE):
        b_ = c // 4
        q0 = 512 * (c % 4)
        w0 = 1536 - q0
        tsl = slice(T * b_, T * b_ + T)

        # zero-padded DoubleRow stationaries/movings (4-head K=256 packing)
        qvT4z = np.zeros((P, NH, 4, 2, P), F8)
        quTz4 = np.zeros((P, NH, 2, Q), F8)
        for h in range(NH):
            hc = h // 2
            r = hc % 2
            p0 = DK * (h % 2)
            qv = qvT_full[p0:p0 + DK, hc, 512 * c:512 * c + 512]
            # target slice axes (p, qt, m) <- qv reshaped (p, qt, m)
            qvT4z[p0:p0 + DK, h, :, r, :] = qv.reshape(DK, 4, P)
            quTz4[p0:p0 + DK, h, r, :] = quT_full[
                p0:p0 + DK, hc, 512 * c:512 * c + 512]

        kT4_c = np.ascontiguousarray(kT_f8[:, :, tsl])

        vv = vT_f8[:, :, tsl]                                     # (128,6,2048)
        vmat = np.ascontiguousarray(
            vv.transpose(1, 0, 2).reshape(H, T))                  # (768,2048)=v.T
        arr = vmat.reshape(NH, DK, 16, P).transpose(0, 3, 2, 1)   # (12,128,16,64)
        vb_c = np.zeros((P, NH, 8, 2, 96), F8)
        vb_c[:, :, :, :, :DK] = arr.reshape(NH, P, 8, 2, DK).transpose(
            1, 0, 2, 3, 4)
        vb_c[:, :, :, :, DK] = 1.0
        entry = {
            "qvT4z": qvT4z,
            "pTw": np.ascontiguousarray(pT_f8[:, :, w0:w0 + WWIN]),
            "quTz4": quTz4,
            "kT4": kT4_c,
            "vb2": vb_c,
            "xq": np.ascontiguousarray(
                hf[T * b_ + q0: T * b_ + q0 + 512].reshape(4, P, H)
                .transpose(1, 0, 2)),
            "Wo": wo_c, "W1": w1_c, "W2": w2_c, "b1c": b1_c,
        }
        if affine:
            entry["vecs"] = vecs
        if use_mask:
            m = f32(mask[b_])
            mT = m.T[:, q0:q0 + 512]                              # (2048,512) j,q
            entry["maskb"] = (mT.reshape(16, P, 512).transpose(1, 0, 2)
                              * np.float32(-240.0)).astype(F8)
            entry["keepb"] = (1.0 - mT.reshape(16, P, 512)
                              .transpose(1, 0, 2)).astype(F8)
        in2.append(entry)

    PROFILE["in2"] = in2
    _res2 = run_bass_kernel_spmd(d2, in2, core_ids=list(range(NCORE)),
                                 trace=_trace)
    PROFILE["d2_ns"] = _res2.exec_time_ns
    PROFILE["d2_res"] = _res2
    r2 = _res2.results

    outp = np.zeros((B, T, H), np.float32)
    for c in range(NCORE):
        b_ = c // 4
        q0 = 512 * (c % 4)
        outp[b_, q0:q0 + 512] = r2[c]["out"].transpose(1, 0, 2).reshape(512, H)
    return outp


# revision 22
# speedup vs baseline: 1.1412x; 1.1412x over previous
"""Trainium2 Bass kernel for a Transformer-XL style BertLayer (relative attention).

Sharding (8 NeuronCores, full inputs in / full output out):
  Dispatch 1: token-sharded transposed projections qT (bf16) / kT/vT/pT (fp8).
  Host: reassemble; add pos_bias_u/v; build fp8 operands; query-split for
    dispatch 2 (core c: batch c//4, queries [512*(c%4), +512)).
  Dispatch 2: attention with keys-on-partitions. All score/FFN matmuls use
    fp8 DoubleRow (0.5 cyc/col):
      - BD rect: 4-head zero-padded stationary qvT4z against dense pT pairs.
      - content: dense kT pair stationary against per-head zero-padded quTz4
        moving (zero rows of other heads contribute nothing).
      - rel-shift via DRAM rect + sheared fp8->f32 casting SWDGE readback,
        PE-transposed (f32r) into the content-score PSUM accumulation.
      - softmax denominators ride as a ones-column in V (fp8 DR); all 12
        head denominators are reciprocal'd in one batched DVE op at the end.
      - FFN in fp8 DoubleRow (W1/W2/x1T/h1 fp8), FFN2 accumulated fully in
        PSUM per query block with LN2 overlapped per block.
"""

import os
import sys
import numpy as np
import ml_dtypes

sys.path.insert(0, "/opt/trn_rl_repo")

import concourse.bass as bass
import concourse.mybir as mybir
import concourse.tile as tile
from concourse import bacc
from concourse.bass_utils import run_bass_kernel_spmd
from concourse.masks import make_identity

BF = ml_dtypes.bfloat16
F8 = ml_dtypes.float8_e4m3
F32, BF16, F32R = mybir.dt.float32, mybir.dt.bfloat16, mybir.dt.float32r
FP8 = mybir.dt.float8e4
DR = mybir.MatmulPerfMode.DoubleRow
AFT = mybir.ActivationFunctionType
ALU = mybir.AluOpType
AXX = mybir.AxisListType.X

B, T, H, NH, DK = 2, 2048, 768, 12, 64
P = 128
FC = H // P            # 6 feature chunks
GC = 3072 // P         # 24 intermediate chunks
Q = 512                # queries per core
NCORE = 8
WWIN = 2560            # pT window width per core
BDW = 2176             # BD rect row width (2175 used + 1 pad)
LN_EPS = 1e-5
WSC = 16.0             # fp8 weight pre-scale (host side)
CSC = 32.0             # ctxT pre-scale

_cache = {}
PROFILE = {}


def _build_d1():
    nc = bacc.Bacc(None, target_bir_lowering=False)
    xT = nc.dram_tensor("xT", [P, FC, Q], FP8, kind="ExternalInput")
    posT = nc.dram_tensor("posT", [P, FC, Q], FP8, kind="ExternalInput")
    ws = {n: nc.dram_tensor(n, [P, FC, 3, 2, P], FP8, kind="ExternalInput")
          for n in ("Wq", "Wk", "Wv", "Wp")}
    bs = {n: nc.dram_tensor(n, [P, FC], F32, kind="ExternalInput")
          for n in ("bq", "bk", "bv")}
    odt = {"qT": BF16, "kT": FP8, "vT": FP8, "pT": FP8}
    outs = {n: nc.dram_tensor(n, [P, FC, Q], odt[n], kind="ExternalOutput")
            for n in ("qT", "kT", "vT", "pT")}

    with tile.TileContext(nc) as tc:
        with tc.tile_pool(name="sb", bufs=2) as sb, \
             tc.tile_pool(name="wp", bufs=2) as wp, \
             tc.tile_pool(name="ps", bufs=3, space="PSUM") as psp:
            xT_sb = sb.tile([P, FC, Q], FP8, tag="x")
            nc.sync.dma_start(xT_sb[:], xT[:])
            posT_sb = sb.tile([P, FC, Q], FP8, tag="p")
            nc.scalar.dma_start(posT_sb[:], posT[:])
            bias_sb = {}
            for n in bs:
                t = sb.tile([P, FC], F32, tag=n)
                nc.scalar.dma_start(t[:], bs[n][:])
                bias_sb[n] = t

            ev = 0
            for wn, bn, on, src in (("Wq", "bq", "qT", xT_sb),
                                    ("Wk", "bk", "kT", xT_sb),
                                    ("Wv", "bv", "vT", xT_sb),
                                    ("Wp", None, "pT", posT_sb)):
                w_sb = wp.tile([P, FC, 3, 2, P], FP8, tag="w")
                (nc.sync if ev % 2 == 0 else nc.scalar).dma_start(
                    w_sb[:], ws[wn][:])
                ev += 1
                o_sb = sb.tile([P, FC, Q], odt[on], tag="o" + on)
                for dc in range(FC):
                    ps = psp.tile([P, Q], F32, tag="ps")
                    for fcp in range(3):
                        nc.tensor.matmul(ps[:], w_sb[:, dc, fcp],
                                         src[:, 2 * fcp:2 * fcp + 2, :],
                                         perf_mode=DR,
                                         start=(fcp == 0), stop=(fcp == 2))
                    if bn is None:
                        nc.scalar.activation(o_sb[:, dc], ps[:], AFT.Copy,
                                             scale=1.0 / WSC)
                    else:
                        nc.scalar.activation(o_sb[:, dc], ps[:], AFT.Identity,
                                             scale=1.0 / WSC,
                                             bias=bias_sb[bn][:, dc:dc + 1])
                nc.sync.dma_start(outs[on][:], o_sb[:])
    nc.compile()
    return nc


def _build_d2(use_mask: bool, affine: bool):
    nc = bacc.Bacc(None, target_bir_lowering=False)
    qvT4z = nc.dram_tensor("qvT4z", [P, NH, 4, 2, P], FP8, kind="ExternalInput")
    pTw = nc.dram_tensor("pTw", [P, FC, WWIN], FP8, kind="ExternalInput")
    quTz4 = nc.dram_tensor("quTz4", [P, NH, 2, Q], FP8, kind="ExternalInput")
    kT4 = nc.dram_tensor("kT4", [P, FC, T], FP8, kind="ExternalInput")
    vb2 = nc.dram_tensor("vb2", [P, NH, 8, 2, 96], FP8, kind="ExternalInput")
    Wo = nc.dram_tensor("Wo", [P, FC, H], FP8, kind="ExternalInput")
    W1 = nc.dram_tensor("W1", [P, FC, 3072], BF16, kind="ExternalInput")
    W2 = nc.dram_tensor("W2", [P, GC, H], BF16, kind="ExternalInput")
    b1c = nc.dram_tensor("b1c", [P, GC], F32, kind="ExternalInput")
    xq = nc.dram_tensor("xq", [P, 4, H], F32, kind="ExternalInput")
    if affine:
        # rows: 0=bo 1=b2 2=ln1_g 3=ln1_b 4=ln2_g 5=ln2_b
        vecs = nc.dram_tensor("vecs", [P, 6, H], F32, kind="ExternalInput")
    if use_mask:
        maskb = nc.dram_tensor("maskb", [P, 16, Q], FP8, kind="ExternalInput")
        keepb = nc.dram_tensor("keepb", [P, 16, Q], FP8, kind="ExternalInput")
    out = nc.dram_tensor("out", [P, 4, H], F32, kind="ExternalOutput")

    wb = 1 if (use_mask or affine) else 2
    shb = 1 if affine else 2
    with tile.TileContext(nc) as tc:
        with tc.tile_pool(name="res", bufs=1) as res, \
             tc.tile_pool(name="stream", bufs=3) as stream, \
             tc.tile_pool(name="work", bufs=2) as work, \
             tc.tile_pool(name="dram", bufs=4, space="DRAM") as dpool:

            # ---------------- resident loads (attention-critical first) ----
            qvT4z_sb = res.tile([P, NH, 4, 2, P], FP8, tag="qvT4z")
            nc.sync.dma_start(qvT4z_sb[:, 0:4], qvT4z[:, 0:4])
            pT_sb = res.tile([P, FC, WWIN], FP8, tag="pTw")
            nc.sync.dma_start(pT_sb[:, 0:2], pTw[:, 0:2])
            kT4_sb = res.tile([P, FC, T], FP8, tag="kT4")
            nc.sync.dma_start(kT4_sb[:, 0:2], kT4[:, 0:2])
            quTz4_sb = res.tile([P, NH, 2, Q], FP8, tag="quTz4")
            nc.sync.dma_start(quTz4_sb[:, 0:4], quTz4[:, 0:4])
            vb_h0 = stream.tile([P, 8, 2, 96], FP8, tag="vbh", bufs=2)
            nc.sync.dma_start(vb_h0[:], vb2[:, 0])
            nc.sync.dma_start(qvT4z_sb[:, 4:], qvT4z[:, 4:])
            nc.sync.dma_start(pT_sb[:, 2:], pTw[:, 2:])
            nc.sync.dma_start(kT4_sb[:, 2:], kT4[:, 2:])
            nc.sync.dma_start(quTz4_sb[:, 4:], quTz4[:, 4:])
            if use_mask:
                mb_sb = res.tile([P, 16, Q], FP8, tag="maskb")
                nc.scalar.dma_start(mb_sb[:], maskb[:])
                kb_sb = res.tile([P, 16, Q], FP8, tag="keepb")
                nc.scalar.dma_start(kb_sb[:], keepb[:])
            Wo_sb = res.tile([P, FC, H], FP8, tag="Wo")
            nc.scalar.dma_start(Wo_sb[:], Wo[:])
            b1_sb = res.tile([P, GC], F32, tag="b1c")
            nc.scalar.dma_start(b1_sb[:], b1c[:])
            if affine:
                vecs_sb = res.tile([P, 6, H], F32, tag="vecs")
                nc.scalar.dma_start(vecs_sb[:], vecs[:])

            ident_raw = res.tile([P, P], F32, tag="idraw")
            make_identity(nc, ident_raw[:])
            ident = res.tile([P, P], F32R, tag="ident")
            nc.vector.tensor_copy(out=ident[:], in_=ident_raw[:])
            ident_bf = res.tile([P, P], BF16, tag="identbf")
            nc.vector.tensor_copy(out=ident_bf[:], in_=ident_raw[:])
            eps_sb = res.tile([P, 1], F32, tag="eps")
            nc.any.memset(eps_sb[:], LN_EPS)
            ones_b = res.tile([1, DK], BF16, tag="onesb")
            nc.any.memset(ones_b[:], CSC)

            ctxT = res.tile([P, FC, Q], FP8, tag="ctxT")
            ctxTf = res.tile([P, FC, Q], FP8, tag="ctxTf")
            den_b = [res.tile([1, Q], BF16, tag=f"denb{h}",
                              name=f"denb{h}") for h in range(NH)]

            # ---------------- attention ----------------
            with tc.tile_pool(name="ps_bd", bufs=3, space="PSUM") as ps_bd, \
                 tc.tile_pool(name="ps_st", bufs=2, space="PSUM") as ps_st, \
                 tc.tile_pool(name="ps_ctx", bufs=1, space="PSUM") as ps_ctx:
                def stage_A_qt(h, bdd, qt):
                    cg = 2 * (h // 4)
                    loc = 384 - 128 * qt
                    bd_sb = work.tile([P, BDW], FP8, tag="bd_sb", bufs=3)
                    for rc in range(5):
                        w = 512 if rc < 4 else 128
                        ps = ps_bd.tile([P, 512], F32, tag="bd")
                        nc.tensor.matmul(
                            ps[:, :w],
                            qvT4z_sb[:, h, qt],
                            pT_sb[:, cg:cg + 2,
                                  loc + rc * 512: loc + rc * 512 + w],
                            perf_mode=DR, start=True, stop=True)
                        if rc in (1, 4):
                            nc.scalar.activation(
                                bd_sb[:, rc * 512: rc * 512 + w],
                                ps[:, :w], AFT.Copy)
                        else:
                            nc.vector.tensor_copy(
                                out=bd_sb[:, rc * 512: rc * 512 + w],
                                in_=ps[:, :w])
                    nc.sync.dma_start(bdd[qt], bd_sb[:])

                def stage_B(h, bdd, vb_h, bdd_next):
                    hp, hc = DK * (h % 2), h // 2
                    cg = 2 * (h // 4)
                    ctx = ps_ctx.tile([96, Q], F32, tag="ctx")
                    for jq in range(4):
                        if bdd_next is not None:
                            stage_A_qt(h + 1, bdd_next, jq)
                        bdsh = stream.tile([P, 4, 512], F32R, tag="bdsh",
                                           bufs=shb)
                        src = bass.AP(bdd.tensor, bdd.offset + 127 + 512 * jq,
                                      [[BDW - 1, P], [P * BDW, 4], [1, 512]])
                        nc.gpsimd.dma_start(bdsh[:], src)
                        for jph in range(2):
                            jp = 2 * jq + jph
                            st = ps_st.tile([P, 1024], F32, tag="st")
                            for half in range(2):
                                jt = 2 * jp + half
                                co = 512 * half
                                for qt in range(4):
                                    nc.tensor.matmul(
                                        st[:, co + qt * P: co + (qt + 1) * P
                                           ].bitcast(F32R),
                                        bdsh[:, qt,
                                             jph * 256 + half * P:
                                             jph * 256 + (half + 1) * P],
                                        ident[:], is_transpose=True,
                                        start=(qt == 0), stop=False)
                                nc.tensor.matmul(
                                    st[:, co:co + 512],
                                    kT4_sb[:, cg:cg + 2,
                                           jt * P:(jt + 1) * P],
                                    quTz4_sb[:, h],
                                    perf_mode=DR, start=False, stop=True)
                                if use_mask:
                                    nc.vector.tensor_tensor(
                                        st[:, co:co + 512], st[:, co:co + 512],
                                        mb_sb[:, jt, :], ALU.add)
                            e2 = work.tile([P, 2, Q], FP8, tag="e2",
                                           bufs=2)
                            nc.scalar.activation(e2[:], st[:],
                                                 AFT.Exp, scale=0.125)
                            if use_mask:
                                for half in range(2):
                                    jt = 2 * jp + half
                                    nc.vector.tensor_tensor(
                                        e2[:, half], e2[:, half],
                                        kb_sb[:, jt, :], ALU.mult)
                            nc.tensor.matmul(ctx[:], vb_h[:, jp], e2[:],
                                             perf_mode=DR,
                                             start=(jp == 0), stop=(jp == 7))
                    # stash ctx fp8 + bf16 reciprocal row
                    nc.vector.tensor_copy(out=ctxTf[hp:hp + DK, hc, :],
                                          in_=ctx[:DK, :])
                    with nc.allow_low_precision(reason="denominator bf16"):
                        nc.vector.reciprocal(den_b[h][:], ctx[DK:DK + 1, :])

                bdd_h = dpool.tile([4, P, BDW], FP8, tag="bdd")
                for qt in range(4):
                    stage_A_qt(0, bdd_h, qt)
                for h in range(NH):
                    if h == 0:
                        vb_h = vb_h0
                    else:
                        vb_h = stream.tile([P, 8, 2, 96], FP8, tag="vbh",
                                           bufs=2)
                        nc.sync.dma_start(vb_h[:], vb2[:, h])
                    if h + 1 < NH:
                        bdd_next = dpool.tile([4, P, BDW], FP8, tag="bdd")
                    else:
                        bdd_next = None
                    stage_B(h, bdd_h, vb_h, bdd_next)
                    bdd_h = bdd_next

            # ---------------- Wo + LN1 + x1T ----------------
            x1f = res.tile([P, 4, H], F32, tag="x1f")
            x1T = res.tile([P, FC, Q], BF16, tag="x1T")

            def layer_norm(dst, r, s1, g_row, b_row, bf_copy=None):
                # r: (P, H) f32 work AP; s1: (P, 1) row-sum of r
                nm = work.tile([P, 1], F32, tag="nm")
                nc.scalar.mul(nm[:], s1, -1.0 / H)
                nc.vector.tensor_scalar(r, r, nm[:], None, op0=ALU.add)
                sq = work.tile([P, H], F32, tag="sq", bufs=wb)
                s2 = work.tile([P, 1], F32, tag="s2")
                nc.vector.scalar_tensor_tensor(
                    sq[:], r, 0.0, r, op0=ALU.add, op1=ALU.mult,
                    accum_out=s2[:])
                sd = work.tile([P, 1], F32, tag="sd")
                nc.scalar.activation(sd[:], s2[:], AFT.Sqrt, scale=1.0 / H,
                                     bias=eps_sb[:, 0:1])
                rstd = work.tile([P, 1], F32, tag="rstd")
                nc.vector.reciprocal(rstd[:], sd[:])
                if affine:
                    nc.vector.tensor_scalar(r, r, rstd[:], None, op0=ALU.mult)
                    nc.vector.tensor_tensor(sq[:], r, g_row, ALU.mult)
                    nc.vector.tensor_tensor(dst, sq[:], b_row, ALU.add)
                else:
                    nc.vector.tensor_scalar(dst, r, rstd[:], None,
                                            op0=ALU.mult)
                if bf_copy is not None:
                    nc.vector.tensor_copy(out=bf_copy, in_=dst)

            with tc.tile_pool(name="ps_ao", bufs=2, space="PSUM") as ps_ao, \
                 tc.tile_pool(name="ps_tr", bufs=2, space="PSUM") as ps_tr, \
                 tc.tile_pool(name="ps_bc", bufs=2, space="PSUM") as ps_bc:
                for h in range(NH):
                    hp, hc = DK * (h % 2), h // 2
                    bc = ps_bc.tile([DK, Q], F32, tag="bc")
                    nc.tensor.matmul(bc[:], ones_b[:], den_b[h][:],
                                     start=True, stop=True)
                    nc.vector.tensor_tensor(ctxT[hp:hp + DK, hc, :],
                                            ctxTf[hp:hp + DK, hc, :], bc[:],
                                            ALU.mult)
                for qt in range(4):
                    ao = ps_ao.tile([P, H], F32, tag="ao")
                    for hcp in range(FC // 2):
                        for c0, c1 in ((0, 512), (512, 768)):
                            nc.tensor.matmul(
                                ao[:, c0:c1],
                                ctxT[:, 2 * hcp:2 * hcp + 2,
                                     qt * P:(qt + 1) * P],
                                Wo_sb[:, 2 * hcp:2 * hcp + 2, c0:c1],
                                perf_mode=DR,
                                start=(hcp == 0), stop=(hcp == FC // 2 - 1))
                    xqt = stream.tile([P, H], F32, tag="xqt", bufs=2)
                    nc.scalar.dma_start(xqt[:], xq[:, qt])
                    resid = work.tile([P, H], F32, tag="resid")
                    s1 = work.tile([P, 1], F32, tag="s1")
                    nc.vector.scalar_tensor_tensor(
                        resid[:], ao[:], 1.0 / (WSC * CSC), xqt[:],
                        op0=ALU.mult, op1=ALU.add,
                        accum_out=None if affine else s1[:])
                    if affine:
                        nc.vector.scalar_tensor_tensor(
                            resid[:], resid[:], 1.0, vecs_sb[:, 0],
                            op0=ALU.mult, op1=ALU.add, accum_out=s1[:])
                    x1b = work.tile([P, H], BF16, tag="x1b")
                    layer_norm(x1f[:, qt], resid[:], s1[:],
                               vecs_sb[:, 2] if affine else None,
                               vecs_sb[:, 3] if affine else None,
                               bf_copy=x1b[:])
                    tp = ps_tr.tile([P, H], BF16, tag="tr")
                    for fcc in range(FC):
                        nc.tensor.matmul(tp[:, fcc * P:(fcc + 1) * P],
                                         x1b[:, fcc * P:(fcc + 1) * P],
                                         ident_bf[:], is_transpose=True,
                                         start=(fcc == 0), stop=(fcc == 5))
                    nc.vector.tensor_copy(
                        out=x1T[:, :, qt * P:(qt + 1) * P], in_=tp[:])

            # ---------------- FFN (bf16) ----------------
            h1_tiles = [res.tile([P, Q], BF16, tag=f"h1_{gc}",
                                 name=f"h1_{gc}") for gc in range(GC)]
            o_acc = [res.tile([P, H], BF16, tag=f"oa{qt}",
                              name=f"oa{qt}") for qt in range(4)]
            with tc.tile_pool(name="ps_h1", bufs=2, space="PSUM") as ps_h1, \
                 tc.tile_pool(name="ps_o2", bufs=2, space="PSUM") as ps_o2:
                for cc in range(4):
                    w2c = stream.tile([P, 6, H], BF16, tag="w2c", bufs=2)
                    nc.sync.dma_start(w2c[:], W2[:, cc * 6:(cc + 1) * 6, :])
                    for gi in range(6):
                        gc = cc * 6 + gi
                        w1s = stream.tile([P, FC, P], BF16, tag="w1s",
                                          bufs=3)
                        nc.sync.dma_start(w1s[:],
                                          W1[:, :, gc * P:(gc + 1) * P])
                        hp1 = ps_h1.tile([P, Q], F32, tag="h1")
                        for fc in range(FC):
                            nc.tensor.matmul(hp1[:], w1s[:, fc],
                                             x1T[:, fc, :],
                                             start=(fc == 0),
                                             stop=(fc == FC - 1))
                        nc.scalar.activation(h1_tiles[gc][:], hp1[:],
                                             AFT.Gelu,
                                             bias=b1_sb[:, gc:gc + 1])
                    for qt in range(4):
                        po = ps_o2.tile([P, 1024], F32, tag="po")
                        for gi in range(6):
                            gc = cc * 6 + gi
                            for c0, c1 in ((0, 512), (512, 768)):
                                nc.tensor.matmul(
                                    po[:, c0:c1],
                                    h1_tiles[gc][:, qt * P:(qt + 1) * P],
                                    w2c[:, gi, c0:c1],
                                    start=(gi == 0), stop=(gi == 5))
                        if cc == 0:
                            nc.vector.tensor_copy(out=o_acc[qt][:],
                                                  in_=po[:, :H])
                        else:
                            nc.vector.tensor_tensor(o_acc[qt][:],
                                                    o_acc[qt][:],
                                                    po[:, :H], ALU.add)
                for qt in range(4):
                    r2 = work.tile([P, H], F32, tag="resid2", bufs=wb)
                    s1b = work.tile([P, 1], F32, tag="s1b")
                    nc.vector.scalar_tensor_tensor(
                        r2[:], o_acc[qt][:], 1.0, x1f[:, qt],
                        op0=ALU.mult, op1=ALU.add,
                        accum_out=None if affine else s1b[:])
                    if affine:
                        nc.vector.scalar_tensor_tensor(
                            r2[:], r2[:], 1.0, vecs_sb[:, 1],
                            op0=ALU.mult, op1=ALU.add, accum_out=s1b[:])
                    o_sb = work.tile([P, H], F32, tag="osb")
                    layer_norm(o_sb[:], r2[:], s1b[:],
                               vecs_sb[:, 4] if affine else None,
                               vecs_sb[:, 5] if affine else None)
                    nc.sync.dma_start(out[:, qt], o_sb[:])
    nc.compile()
    return nc


# --------------------------------------------------------------------------
def _chunk_pf(w):
    """(768, X) -> (128, 6, X) with row d' = 128*chunk + partition."""
    return np.ascontiguousarray(w.reshape(FC, P, -1).transpose(1, 0, 2))


def kernel(hidden_states, attention_mask, pos_emb,
           Wq, bq, Wk, bk, Wv, bv, Wp, pos_bias_u, pos_bias_v, Wo, bo,
           ln1_g, ln1_b, W1, b1, W2, b2, ln2_g, ln2_b):
    f32 = lambda x: np.asarray(x, dtype=np.float32)
    hidden_states = f32(hidden_states)
    pos_emb = f32(pos_emb)
    mask = np.asarray(attention_mask)
    use_mask = bool(mask.any())
    affine = not (np.all(f32(ln1_g) == 1) and np.all(f32(ln1_b) == 0)
                  and np.all(f32(ln2_g) == 1) and np.all(f32(ln2_b) == 0)
                  and np.all(f32(bo) == 0) and np.all(f32(b2) == 0))

    if "d1" not in _cache:
        _cache["d1"] = _build_d1()
    key = ("d2", use_mask, affine)
    if key not in _cache:
        _cache[key] = _build_d2(use_mask, affine)
    d1, d2 = _cache["d1"], _cache[key]

    hf = hidden_states.reshape(B * T, H)
    xT_full = _chunk_pf(np.ascontiguousarray(hf.T)).astype(F8)
    posT_pad = np.zeros((H, 4096), np.float32)
    posT_pad[:, :2 * T - 1] = pos_emb[0].T
    posT_full = _chunk_pf(posT_pad).astype(F8)

    def _w_d1(w):
        # [p, dc, fcp, t, m] = WSC * w[256*fcp + 128*t + p, 128*dc + m]
        return np.ascontiguousarray(
            (f32(w) * WSC).reshape(3, 2, P, FC, P).transpose(2, 3, 0, 1, 4)
        ).astype(F8)

    wq_c, wk_c, wv_c, wp_c = map(_w_d1, (Wq, Wk, Wv, Wp))
    bq_c = f32(bq).reshape(FC, P).T.copy()
    bk_c = f32(bk).reshape(FC, P).T.copy()
    bv_c = f32(bv).reshape(FC, P).T.copy()

    in1 = []
    for c in range(NCORE):
        sl = slice(512 * c, 512 * c + 512)
        in1.append({
            "xT": np.ascontiguousarray(xT_full[:, :, sl]),
            "posT": np.ascontiguousarray(posT_full[:, :, sl]),
            "Wq": wq_c, "Wk": wk_c, "Wv": wv_c, "Wp": wp_c,
            "bq": bq_c, "bk": bk_c, "bv": bv_c,
        })
    _trace = bool(os.environ.get("BERT_KERNEL_TRACE"))
    _res1 = run_bass_kernel_spmd(d1, in1, core_ids=list(range(NCORE)),
                                 trace=_trace)
    PROFILE["d1_ns"] = _res1.exec_time_ns
    PROFILE["d1_res"] = _res1
    r1 = _res1.results

    qT_full = np.concatenate([r["qT"] for r in r1], axis=2).astype(np.float32)
    kT_f8 = np.concatenate([r["kT"] for r in r1], axis=2)
    vT_f8 = np.concatenate([r["vT"] for r in r1], axis=2)
    pT_f8 = np.concatenate([r["pT"] for r in r1], axis=2)
    pT_f8[:, :, 2 * T - 1:] = 0

    pbu_c = f32(pos_bias_u).reshape(NH * DK).reshape(FC, P).T.copy()
    pbv_c = f32(pos_bias_v).reshape(NH * DK).reshape(FC, P).T.copy()
    quT_full = (qT_full + pbu_c[:, :, None]).astype(F8)
    qvT_full = (qT_full + pbv_c[:, :, None]).astype(F8)

    wo_c = (_chunk_pf(f32(Wo)) * WSC).astype(F8)
    w1_c = _chunk_pf(f32(W1)).astype(BF)
    w2_c = np.ascontiguousarray(
        f32(W2).reshape(GC, P, H).transpose(1, 0, 2)).astype(BF)
    b1_c = f32(b1).reshape(GC, P).T.copy()
    if affine:
        vecs = np.stack([np.broadcast_to(f32(x), (P, H)) for x in
                         (bo, b2, ln1_g, ln1_b, ln2_g, ln2_b)], axis=1).copy()

    in2 = []
    for c in range(NCORE):
        b_ = c // 4
        q0 = 512 * (c % 4)
        w0 = 1536 - q0
        tsl = slice(T * b_, T * b_ + T)

        # zero-padded DoubleRow stationaries/movings (4-head K=256 packing)
        qvT4z = np.zeros((P, NH, 4, 2, P), F8)
        quTz4 = np.zeros((P, NH, 2, Q), F8)
        for h in range(NH):
            hc = h // 2
            r = hc % 2
            p0 = DK * (h % 2)
            qv = qvT_full[p0:p0 + DK, hc, 512 * c:512 * c + 512]
            # target slice axes (p, qt, m) <- qv reshaped (p, qt, m)
            qvT4z[p0:p0 + DK, h, :, r, :] = qv.reshape(DK, 4, P)
            quTz4[p0:p0 + DK, h, r, :] = quT_full[
                p0:p0 + DK, hc, 512 * c:512 * c + 512]

        kT4_c = np.ascontiguousarray(kT_f8[:, :, tsl])

        vv = vT_f8[:, :, tsl]                                     # (128,6,2048)
        vmat = np.ascontiguousarray(
            vv.transpose(1, 0, 2).reshape(H, T))                  # (768,2048)=v.T
        arr = vmat.reshape(NH, DK, 16, P).transpose(0, 3, 2, 1)   # (12,128,16,64)
        vb_c = np.zeros((P, NH, 8, 2, 96), F8)
        vb_c[:, :, :, :, :DK] = arr.reshape(NH, P, 8, 2, DK).transpose(
            1, 0, 2, 3, 4)
        vb_c[:, :, :, :, DK] = 1.0
        entry = {
            "qvT4z": qvT4z,
            "pTw": np.ascontiguousarray(pT_f8[:, :, w0:w0 + WWIN]),
            "quTz4": quTz4,
            "kT4": kT4_c,
            "vb2": vb_c,
            "xq": np.ascontiguousarray(
                hf[T * b_ + q0: T * b_ + q0 + 512].reshape(4, P, H)
                .transpose(1, 0, 2)),
            "Wo": wo_c, "W1": w1_c, "W2": w2_c, "b1c": b1_c,
        }
        if affine:
            entry["vecs"] = vecs
        if use_mask:
            m = f32(mask[b_])
            mT = m.T[:, q0:q0 + 512]                              # (2048,512) j,q
            entry["maskb"] = (mT.reshape(16, P, 512).transpose(1, 0, 2)
                              * np.float32(-240.0)).astype(F8)
            entry["keepb"] = (1.0 - mT.reshape(16, P, 512)
                              .transpose(1, 0, 2)).astype(F8)
        in2.append(entry)

    PROFILE["in2"] = in2
    _res2 = run_bass_kernel_spmd(d2, in2, core_ids=list(range(NCORE)),
                                 trace=_trace)
    PROFILE["d2_ns"] = _res2.exec_time_ns
    PROFILE["d2_res"] = _res2
    r2 = _res2.results

    outp = np.zeros((B, T, H), np.float32)
    for c in range(NCORE):
        b_ = c // 4
        q0 = 512 * (c % 4)
        outp[b_, q0:q0 + 512] = r2[c]["out"].transpose(1, 0, 2).reshape(512, H)
    return outp


# revision 24
# speedup vs baseline: 1.1718x; 1.0269x over previous
"""Trainium2 Bass kernel for a Transformer-XL style BertLayer (relative attention).

Sharding (8 NeuronCores, full inputs in / full output out):
  Dispatch 1: token-sharded transposed projections qT (bf16) / kT/vT/pT (fp8).
  Host: reassemble; add pos_bias_u/v; build fp8 operands; query-split for
    dispatch 2 (core c: batch c//4, queries [512*(c%4), +512)).
  Dispatch 2: attention with keys-on-partitions. All score/FFN matmuls use
    fp8 DoubleRow (0.5 cyc/col):
      - BD rect: 4-head zero-padded stationary qvT4z against dense pT pairs.
      - content: dense kT pair stationary against per-head zero-padded quTz4
        moving (zero rows of other heads contribute nothing).
      - rel-shift via DRAM rect + sheared fp8->f32 casting SWDGE readback,
        PE-transposed (f32r) into the content-score PSUM accumulation.
      - softmax denominators ride as a ones-column in V (fp8 DR); all 12
        head denominators are reciprocal'd in one batched DVE op at the end.
      - FFN in fp8 DoubleRow (W1/W2/x1T/h1 fp8), FFN2 accumulated fully in
        PSUM per query block with LN2 overlapped per block.
"""

import os
import sys
import numpy as np
import ml_dtypes

sys.path.insert(0, "/opt/trn_rl_repo")

import concourse.bass as bass
import concourse.mybir as mybir
import concourse.tile as tile
from concourse import bacc
from concourse.bass_utils import run_bass_kernel_spmd
from concourse.masks import make_identity

BF = ml_dtypes.bfloat16
F8 = ml_dtypes.float8_e4m3
F32, BF16, F32R = mybir.dt.float32, mybir.dt.bfloat16, mybir.dt.float32r
FP8 = mybir.dt.float8e4
DR = mybir.MatmulPerfMode.DoubleRow
AFT = mybir.ActivationFunctionType
ALU = mybir.AluOpType
AXX = mybir.AxisListType.X

B, T, H, NH, DK = 2, 2048, 768, 12, 64
P = 128
FC = H // P            # 6 feature chunks
GC = 3072 // P         # 24 intermediate chunks
Q = 512                # queries per core
NCORE = 8
WWIN = 2560            # pT window width per core
BDW = 2176             # BD rect row width (2175 used + 1 pad)
LN_EPS = 1e-5
WSC = 16.0             # fp8 weight pre-scale (host side)
CSC = 32.0             # ctxT pre-scale

_cache = {}
PROFILE = {}


def _build_d1():
    nc = bacc.Bacc(None, target_bir_lowering=False)
    xT = nc.dram_tensor("xT", [P, FC, Q], FP8, kind="ExternalInput")
    posT = nc.dram_tensor("posT", [P, FC, Q], FP8, kind="ExternalInput")
    ws = {n: nc.dram_tensor(n, [P, FC, 3, 2, P], FP8, kind="ExternalInput")
          for n in ("Wq", "Wk", "Wv", "Wp")}
    bs = {n: nc.dram_tensor(n, [P, FC], F32, kind="ExternalInput")
          for n in ("bq", "bk", "bv")}
    odt = {"qT": BF16, "kT": FP8, "vT": FP8, "pT": FP8}
    outs = {n: nc.dram_tensor(n, [P, FC, Q], odt[n], kind="ExternalOutput")
            for n in ("qT", "kT", "vT", "pT")}

    with tile.TileContext(nc) as tc:
        with tc.tile_pool(name="sb", bufs=2) as sb, \
             tc.tile_pool(name="wp", bufs=2) as wp, \
             tc.tile_pool(name="ps", bufs=3, space="PSUM") as psp:
            xT_sb = sb.tile([P, FC, Q], FP8, tag="x")
            nc.sync.dma_start(xT_sb[:], xT[:])
            posT_sb = sb.tile([P, FC, Q], FP8, tag="p")
            nc.scalar.dma_start(posT_sb[:], posT[:])
            bias_sb = {}
            for n in bs:
                t = sb.tile([P, FC], F32, tag=n)
                nc.scalar.dma_start(t[:], bs[n][:])
                bias_sb[n] = t

            ev = 0
            for wn, bn, on, src in (("Wq", "bq", "qT", xT_sb),
                                    ("Wk", "bk", "kT", xT_sb),
                                    ("Wv", "bv", "vT", xT_sb),
                                    ("Wp", None, "pT", posT_sb)):
                w_sb = wp.tile([P, FC, 3, 2, P], FP8, tag="w")
                (nc.sync if ev % 2 == 0 else nc.scalar).dma_start(
                    w_sb[:], ws[wn][:])
                ev += 1
                o_sb = sb.tile([P, FC, Q], odt[on], tag="o" + on)
                for dc in range(FC):
                    ps = psp.tile([P, Q], F32, tag="ps")
                    for fcp in range(3):
                        nc.tensor.matmul(ps[:], w_sb[:, dc, fcp],
                                         src[:, 2 * fcp:2 * fcp + 2, :],
                                         perf_mode=DR,
                                         start=(fcp == 0), stop=(fcp == 2))
                    if bn is None:
                        nc.scalar.activation(o_sb[:, dc], ps[:], AFT.Copy,
                                             scale=1.0 / WSC)
                    else:
                        nc.scalar.activation(o_sb[:, dc], ps[:], AFT.Identity,
                                             scale=1.0 / WSC,
                                             bias=bias_sb[bn][:, dc:dc + 1])
                nc.sync.dma_start(outs[on][:], o_sb[:])
    nc.compile()
    return nc


def _build_d2(use_mask: bool, affine: bool):
    nc = bacc.Bacc(None, target_bir_lowering=False)
    qvT4z = nc.dram_tensor("qvT4z", [P, NH, 4, 2, P], FP8, kind="ExternalInput")
    pTw = nc.dram_tensor("pTw", [P, FC, WWIN], FP8, kind="ExternalInput")
    quTz4 = nc.dram_tensor("quTz4", [P, NH, 2, Q], FP8, kind="ExternalInput")
    kT4 = nc.dram_tensor("kT4", [P, FC, T], FP8, kind="ExternalInput")
    vb2 = nc.dram_tensor("vb2", [P, NH, 8, 2, 96], FP8, kind="ExternalInput")
    Wo = nc.dram_tensor("Wo", [P, FC, H], FP8, kind="ExternalInput")
    W1 = nc.dram_tensor("W1", [P, FC, 3072], BF16, kind="ExternalInput")
    W2 = nc.dram_tensor("W2", [P, GC, H], BF16, kind="ExternalInput")
    b1c = nc.dram_tensor("b1c", [P, GC], F32, kind="ExternalInput")
    xq = nc.dram_tensor("xq", [P, 4, H], F32, kind="ExternalInput")
    if affine:
        # rows: 0=bo 1=b2 2=ln1_g 3=ln1_b 4=ln2_g 5=ln2_b
        vecs = nc.dram_tensor("vecs", [P, 6, H], F32, kind="ExternalInput")
    if use_mask:
        maskb = nc.dram_tensor("maskb", [P, 16, Q], FP8, kind="ExternalInput")
        keepb = nc.dram_tensor("keepb", [P, 16, Q], FP8, kind="ExternalInput")
    out = nc.dram_tensor("out", [P, 4, H], F32, kind="ExternalOutput")

    wb = 1 if (use_mask or affine) else 2
    shb = 1 if affine else 2
    with tile.TileContext(nc) as tc:
        with tc.tile_pool(name="res", bufs=1) as res, \
             tc.tile_pool(name="stream", bufs=3) as stream, \
             tc.tile_pool(name="work", bufs=2) as work, \
             tc.tile_pool(name="dram", bufs=4, space="DRAM") as dpool:

            # ---------------- resident loads (attention-critical first) ----
            qvT4z_sb = res.tile([P, NH, 4, 2, P], FP8, tag="qvT4z")
            nc.sync.dma_start(qvT4z_sb[:, 0:4], qvT4z[:, 0:4])
            pT_sb = res.tile([P, FC, WWIN], FP8, tag="pTw")
            nc.sync.dma_start(pT_sb[:, 0:2], pTw[:, 0:2])
            kT4_sb = res.tile([P, FC, T], FP8, tag="kT4")
            nc.sync.dma_start(kT4_sb[:, 0:2], kT4[:, 0:2])
            quTz4_sb = res.tile([P, NH, 2, Q], FP8, tag="quTz4")
            nc.sync.dma_start(quTz4_sb[:, 0:4], quTz4[:, 0:4])
            vb_h0 = stream.tile([P, 8, 2, 96], FP8, tag="vbh", bufs=2)
            nc.sync.dma_start(vb_h0[:], vb2[:, 0])
            nc.sync.dma_start(qvT4z_sb[:, 4:], qvT4z[:, 4:])
            nc.sync.dma_start(pT_sb[:, 2:], pTw[:, 2:])
            nc.sync.dma_start(kT4_sb[:, 2:], kT4[:, 2:])
            nc.sync.dma_start(quTz4_sb[:, 4:], quTz4[:, 4:])
            if use_mask:
                mb_sb = res.tile([P, 16, Q], FP8, tag="maskb")
                nc.scalar.dma_start(mb_sb[:], maskb[:])
                kb_sb = res.tile([P, 16, Q], FP8, tag="keepb")
                nc.scalar.dma_start(kb_sb[:], keepb[:])
            Wo_sb = res.tile([P, FC, H], FP8, tag="Wo")
            nc.scalar.dma_start(Wo_sb[:], Wo[:])
            b1_sb = res.tile([P, GC], F32, tag="b1c")
            nc.scalar.dma_start(b1_sb[:], b1c[:])
            if affine:
                vecs_sb = res.tile([P, 6, H], F32, tag="vecs")
                nc.scalar.dma_start(vecs_sb[:], vecs[:])

            ident_raw = res.tile([P, P], F32, tag="idraw")
            make_identity(nc, ident_raw[:])
            ident = res.tile([P, P], F32R, tag="ident")
            nc.vector.tensor_copy(out=ident[:], in_=ident_raw[:])
            ident_bf = res.tile([P, P], BF16, tag="identbf")
            nc.vector.tensor_copy(out=ident_bf[:], in_=ident_raw[:])
            eps_sb = res.tile([P, 1], F32, tag="eps")
            nc.any.memset(eps_sb[:], LN_EPS)
            ones_b = res.tile([1, DK], BF16, tag="onesb")
            nc.any.memset(ones_b[:], CSC)

            ctxT = res.tile([P, FC, Q], FP8, tag="ctxT")
            ctxTf = res.tile([P, FC, Q], FP8, tag="ctxTf")
            den_b = [res.tile([1, Q], BF16, tag=f"denb{h}",
                              name=f"denb{h}") for h in range(NH)]

            # ---------------- attention ----------------
            with tc.tile_pool(name="ps_bd", bufs=3, space="PSUM") as ps_bd, \
                 tc.tile_pool(name="ps_st", bufs=2, space="PSUM") as ps_st, \
                 tc.tile_pool(name="ps_ctx", bufs=1, space="PSUM") as ps_ctx:
                def stage_A_qt(h, bdd, qt):
                    cg = 2 * (h // 4)
                    loc = 384 - 128 * qt
                    bd_sb = work.tile([P, BDW], FP8, tag="bd_sb", bufs=3)
                    for rc in range(5):
                        w = 512 if rc < 4 else 128
                        ps = ps_bd.tile([P, 512], F32, tag="bd")
                        nc.tensor.matmul(
                            ps[:, :w],
                            qvT4z_sb[:, h, qt],
                            pT_sb[:, cg:cg + 2,
                                  loc + rc * 512: loc + rc * 512 + w],
                            perf_mode=DR, start=True, stop=True)
                        if rc in (1, 4):
                            nc.scalar.activation(
                                bd_sb[:, rc * 512: rc * 512 + w],
                                ps[:, :w], AFT.Copy)
                        else:
                            nc.vector.tensor_copy(
                                out=bd_sb[:, rc * 512: rc * 512 + w],
                                in_=ps[:, :w])
                    nc.sync.dma_start(bdd[qt], bd_sb[:])

                def stage_B(h, bdd, vb_h, bdd_next):
                    hp, hc = DK * (h % 2), h // 2
                    cg = 2 * (h // 4)
                    ctx = ps_ctx.tile([96, Q], F32, tag="ctx")
                    for jq in range(4):
                        if bdd_next is not None:
                            stage_A_qt(h + 1, bdd_next, jq)
                        bdsh = stream.tile([P, 4, 512], F32R, tag="bdsh",
                                           bufs=shb)
                        src = bass.AP(bdd.tensor, bdd.offset + 127 + 512 * jq,
                                      [[BDW - 1, P], [P * BDW, 4], [1, 512]])
                        nc.gpsimd.dma_start(bdsh[:], src)
                        for jph in range(2):
                            jp = 2 * jq + jph
                            st = ps_st.tile([P, 1024], F32, tag="st")
                            for half in range(2):
                                jt = 2 * jp + half
                                co = 512 * half
                                for qt in range(4):
                                    nc.tensor.matmul(
                                        st[:, co + qt * P: co + (qt + 1) * P
                                           ].bitcast(F32R),
                                        bdsh[:, qt,
                                             jph * 256 + half * P:
                                             jph * 256 + (half + 1) * P],
                                        ident[:], is_transpose=True,
                                        start=(qt == 0), stop=False)
                                nc.tensor.matmul(
                                    st[:, co:co + 512],
                                    kT4_sb[:, cg:cg + 2,
                                           jt * P:(jt + 1) * P],
                                    quTz4_sb[:, h],
                                    perf_mode=DR, start=False, stop=True)
                                if use_mask:
                                    nc.vector.tensor_tensor(
                                        st[:, co:co + 512], st[:, co:co + 512],
                                        mb_sb[:, jt, :], ALU.add)
                            e2 = work.tile([P, 2, Q], FP8, tag="e2",
                                           bufs=2)
                            nc.scalar.activation(e2[:], st[:],
                                                 AFT.Exp, scale=0.125)
                            if use_mask:
                                for half in range(2):
                                    jt = 2 * jp + half
                                    nc.vector.tensor_tensor(
                                        e2[:, half], e2[:, half],
                                        kb_sb[:, jt, :], ALU.mult)
                            nc.tensor.matmul(ctx[:], vb_h[:, jp], e2[:],
                                             perf_mode=DR,
                                             start=(jp == 0), stop=(jp == 7))
                    # stash ctx fp8 + den row (frees ctx bank fast),
                    # then reciprocal off SBUF so V(h+1) isn't blocked
                    nc.vector.tensor_copy(out=ctxTf[hp:hp + DK, hc, :],
                                          in_=ctx[:DK, :])
                    den_s = work.tile([1, Q], BF16, tag="den_s", bufs=2)
                    nc.scalar.mul(den_s[:], ctx[DK:DK + 1, :], 1.0)
                    with nc.allow_low_precision(reason="denominator bf16"):
                        nc.vector.reciprocal(den_b[h][:], den_s[:])

                bdd_h = dpool.tile([4, P, BDW], FP8, tag="bdd")
                for qt in range(4):
                    stage_A_qt(0, bdd_h, qt)
                for h in range(NH):
                    if h == 0:
                        vb_h = vb_h0
                    else:
                        vb_h = stream.tile([P, 8, 2, 96], FP8, tag="vbh",
                                           bufs=2)
                        nc.sync.dma_start(vb_h[:], vb2[:, h])
                    if h + 1 < NH:
                        bdd_next = dpool.tile([4, P, BDW], FP8, tag="bdd")
                    else:
                        bdd_next = None
                    stage_B(h, bdd_h, vb_h, bdd_next)
                    bdd_h = bdd_next

            # ---------------- Wo + LN1 + x1T ----------------
            x1f = res.tile([P, 4, H], F32, tag="x1f")
            x1T = res.tile([P, FC, Q], BF16, tag="x1T")

            def layer_norm(dst, r, s1, g_row, b_row, bf_copy=None):
                # r: (P, H) f32 work AP; s1: (P, 1) row-sum of r
                nm = work.tile([P, 1], F32, tag="nm")
                nc.scalar.mul(nm[:], s1, -1.0 / H)
                nc.vector.tensor_scalar(r, r, nm[:], None, op0=ALU.add)
                sq = work.tile([P, H], F32 if affine else BF16,
                               tag="sq", bufs=wb)
                s2 = work.tile([P, 1], F32, tag="s2")
                nc.vector.scalar_tensor_tensor(
                    sq[:], r, 0.0, r, op0=ALU.add, op1=ALU.mult,
                    accum_out=s2[:])
                sd = work.tile([P, 1], F32, tag="sd")
                nc.scalar.activation(sd[:], s2[:], AFT.Sqrt, scale=1.0 / H,
                                     bias=eps_sb[:, 0:1])
                rstd = work.tile([P, 1], F32, tag="rstd")
                nc.vector.reciprocal(rstd[:], sd[:])
                if affine:
                    nc.vector.tensor_scalar(r, r, rstd[:], None, op0=ALU.mult)
                    nc.vector.tensor_tensor(sq[:], r, g_row, ALU.mult)
                    nc.vector.tensor_tensor(dst, sq[:], b_row, ALU.add)
                else:
                    nc.vector.tensor_scalar(dst, r, rstd[:], None,
                                            op0=ALU.mult)
                if bf_copy is not None:
                    nc.vector.tensor_copy(out=bf_copy, in_=dst)

            with tc.tile_pool(name="ps_ao", bufs=2, space="PSUM") as ps_ao, \
                 tc.tile_pool(name="ps_tr", bufs=2, space="PSUM") as ps_tr, \
                 tc.tile_pool(name="ps_bc", bufs=2, space="PSUM") as ps_bc:
                for h in range(NH):
                    hp, hc = DK * (h % 2), h // 2
                    bc = ps_bc.tile([DK, Q], F32, tag="bc")
                    nc.tensor.matmul(bc[:], ones_b[:], den_b[h][:],
                                     start=True, stop=True)
                    nc.vector.tensor_tensor(ctxT[hp:hp + DK, hc, :],
                                            ctxTf[hp:hp + DK, hc, :], bc[:],
                                            ALU.mult)
                for qt in range(4):
                    ao = ps_ao.tile([P, H], F32, tag="ao")
                    for hcp in range(FC // 2):
                        for c0, c1 in ((0, 512), (512, 768)):
                            nc.tensor.matmul(
                                ao[:, c0:c1],
                                ctxT[:, 2 * hcp:2 * hcp + 2,
                                     qt * P:(qt + 1) * P],
                                Wo_sb[:, 2 * hcp:2 * hcp + 2, c0:c1],
                                perf_mode=DR,
                                start=(hcp == 0), stop=(hcp == FC // 2 - 1))
                    xqt = stream.tile([P, H], F32, tag="xqt", bufs=2)
                    nc.scalar.dma_start(xqt[:], xq[:, qt])
                    resid = work.tile([P, H], F32, tag="resid")
                    s1 = work.tile([P, 1], F32, tag="s1")
                    nc.vector.scalar_tensor_tensor(
                        resid[:], ao[:], 1.0 / (WSC * CSC), xqt[:],
                        op0=ALU.mult, op1=ALU.add,
                        accum_out=None if affine else s1[:])
                    if affine:
                        nc.vector.scalar_tensor_tensor(
                            resid[:], resid[:], 1.0, vecs_sb[:, 0],
                            op0=ALU.mult, op1=ALU.add, accum_out=s1[:])
                    x1b = work.tile([P, H], BF16, tag="x1b")
                    layer_norm(x1f[:, qt], resid[:], s1[:],
                               vecs_sb[:, 2] if affine else None,
                               vecs_sb[:, 3] if affine else None,
                               bf_copy=x1b[:])
                    tp = ps_tr.tile([P, H], BF16, tag="tr")
                    for fcc in range(FC):
                        nc.tensor.matmul(tp[:, fcc * P:(fcc + 1) * P],
                                         x1b[:, fcc * P:(fcc + 1) * P],
                                         ident_bf[:], is_transpose=True,
                                         start=(fcc == 0), stop=(fcc == 5))
                    nc.vector.tensor_copy(
                        out=x1T[:, :, qt * P:(qt + 1) * P], in_=tp[:])

            # ---------------- FFN (bf16) ----------------
            h1_tiles = [res.tile([P, Q], BF16, tag=f"h1_{gc}",
                                 name=f"h1_{gc}") for gc in range(GC)]
            o_acc = [res.tile([P, H], BF16, tag=f"oa{qt}",
                              name=f"oa{qt}") for qt in range(4)]
            with tc.tile_pool(name="ps_h1", bufs=2, space="PSUM") as ps_h1, \
                 tc.tile_pool(name="ps_o2", bufs=2, space="PSUM") as ps_o2:
                for cc in range(4):
                    w2c = stream.tile([P, 6, H], BF16, tag="w2c", bufs=2)
                    nc.sync.dma_start(w2c[:], W2[:, cc * 6:(cc + 1) * 6, :])
                    for gi in range(6):
                        gc = cc * 6 + gi
                        w1s = stream.tile([P, FC, P], BF16, tag="w1s",
                                          bufs=3)
                        nc.sync.dma_start(w1s[:],
                                          W1[:, :, gc * P:(gc + 1) * P])
                        hp1 = ps_h1.tile([P, Q], F32, tag="h1")
                        for fc in range(FC):
                            nc.tensor.matmul(hp1[:], w1s[:, fc],
                                             x1T[:, fc, :],
                                             start=(fc == 0),
                                             stop=(fc == FC - 1))
                        nc.scalar.activation(h1_tiles[gc][:], hp1[:],
                                             AFT.Gelu,
                                             bias=b1_sb[:, gc:gc + 1])
                    for qt in range(4):
                        po = ps_o2.tile([P, 1024], F32, tag="po")
                        for gi in range(6):
                            gc = cc * 6 + gi
                            for c0, c1 in ((0, 512), (512, 768)):
                                nc.tensor.matmul(
                                    po[:, c0:c1],
                                    h1_tiles[gc][:, qt * P:(qt + 1) * P],
                                    w2c[:, gi, c0:c1],
                                    start=(gi == 0), stop=(gi == 5))
                        if cc == 0:
                            nc.vector.tensor_copy(out=o_acc[qt][:],
                                                  in_=po[:, :H])
                        else:
                            nc.vector.tensor_tensor(o_acc[qt][:],
                                                    o_acc[qt][:],
                                                    po[:, :H], ALU.add)
                for qt in range(4):
                    r2 = work.tile([P, H], F32, tag="resid2", bufs=wb)
                    s1b = work.tile([P, 1], F32, tag="s1b")
                    nc.vector.scalar_tensor_tensor(
                        r2[:], o_acc[qt][:], 1.0, x1f[:, qt],
                        op0=ALU.mult, op1=ALU.add,
                        accum_out=None if affine else s1b[:])
                    if affine:
                        nc.vector.scalar_tensor_tensor(
                            r2[:], r2[:], 1.0, vecs_sb[:, 1],
                            op0=ALU.mult, op1=ALU.add, accum_out=s1b[:])
                    o_sb = work.tile([P, H], F32, tag="osb")
                    layer_norm(o_sb[:], r2[:], s1b[:],
                               vecs_sb[:, 4] if affine else None,
                               vecs_sb[:, 5] if affine else None)
                    nc.sync.dma_start(out[:, qt], o_sb[:])
    nc.compile()
    return nc


# --------------------------------------------------------------------------
def _chunk_pf(w):
    """(768, X) -> (128, 6, X) with row d' = 128*chunk + partition."""
    return np.ascontiguousarray(w.reshape(FC, P, -1).transpose(1, 0, 2))


def kernel(hidden_states, attention_mask, pos_emb,
           Wq, bq, Wk, bk, Wv, bv, Wp, pos_bias_u, pos_bias_v, Wo, bo,
           ln1_g, ln1_b, W1, b1, W2, b2, ln2_g, ln2_b):
    f32 = lambda x: np.asarray(x, dtype=np.float32)
    hidden_states = f32(hidden_states)
    pos_emb = f32(pos_emb)
    mask = np.asarray(attention_mask)
    use_mask = bool(mask.any())
    affine = not (np.all(f32(ln1_g) == 1) and np.all(f32(ln1_b) == 0)
                  and np.all(f32(ln2_g) == 1) and np.all(f32(ln2_b) == 0)
                  and np.all(f32(bo) == 0) and np.all(f32(b2) == 0))

    if "d1" not in _cache:
        _cache["d1"] = _build_d1()
    key = ("d2", use_mask, affine)
    if key not in _cache:
        _cache[key] = _build_d2(use_mask, affine)
    d1, d2 = _cache["d1"], _cache[key]

    hf = hidden_states.reshape(B * T, H)
    xT_full = _chunk_pf(np.ascontiguousarray(hf.T)).astype(F8)
    posT_pad = np.zeros((H, 4096), np.float32)
    posT_pad[:, :2 * T - 1] = pos_emb[0].T
    posT_full = _chunk_pf(posT_pad).astype(F8)

    def _w_d1(w):
        # [p, dc, fcp, t, m] = WSC * w[256*fcp + 128*t + p, 128*dc + m]
        return np.ascontiguousarray(
            (f32(w) * WSC).reshape(3, 2, P, FC, P).transpose(2, 3, 0, 1, 4)
        ).astype(F8)

    wq_c, wk_c, wv_c, wp_c = map(_w_d1, (Wq, Wk, Wv, Wp))
    bq_c = f32(bq).reshape(FC, P).T.copy()
    bk_c = f32(bk).reshape(FC, P).T.copy()
    bv_c = f32(bv).reshape(FC, P).T.copy()

    in1 = []
    for c in range(NCORE):
        sl = slice(512 * c, 512 * c + 512)
        in1.append({
            "xT": np.ascontiguousarray(xT_full[:, :, sl]),
            "posT": np.ascontiguousarray(posT_full[:, :, sl]),
            "Wq": wq_c, "Wk": wk_c, "Wv": wv_c, "Wp": wp_c,
            "bq": bq_c, "bk": bk_c, "bv": bv_c,
        })
    _trace = bool(os.environ.get("BERT_KERNEL_TRACE"))
    _res1 = run_bass_kernel_spmd(d1, in1, core_ids=list(range(NCORE)),
                                 trace=_trace)
    PROFILE["d1_ns"] = _res1.exec_time_ns
    PROFILE["d1_res"] = _res1
    r1 = _res1.results

    qT_full = np.concatenate([r["qT"] for r in r1], axis=2).astype(np.float32)
    kT_f8 = np.concatenate([r["kT"] for r in r1], axis=2)
    vT_f8 = np.concatenate([r["vT"] for r in r1], axis=2)
    pT_f8 = np.concatenate([r["pT"] for r in r1], axis=2)
    pT_f8[:, :, 2 * T - 1:] = 0

    pbu_c = f32(pos_bias_u).reshape(NH * DK).reshape(FC, P).T.copy()
    pbv_c = f32(pos_bias_v).reshape(NH * DK).reshape(FC, P).T.copy()
    quT_full = (qT_full + pbu_c[:, :, None]).astype(F8)
    qvT_full = (qT_full + pbv_c[:, :, None]).astype(F8)

    wo_c = (_chunk_pf(f32(Wo)) * WSC).astype(F8)
    w1_c = _chunk_pf(f32(W1)).astype(BF)
    w2_c = np.ascontiguousarray(
        f32(W2).reshape(GC, P, H).transpose(1, 0, 2)).astype(BF)
    b1_c = f32(b1).reshape(GC, P).T.copy()
    if affine:
        vecs = np.stack([np.broadcast_to(f32(x), (P, H)) for x in
                         (bo, b2, ln1_g, ln1_b, ln2_g, ln2_b)], axis=1).copy()

    in2 = []
    for c in range(NCORE):
        b_ = c // 4
        q0 = 512 * (c % 4)
        w0 = 1536 - q0
        tsl = slice(T * b_, T * b_ + T)

        # zero-padded DoubleRow stationaries/movings (4-head K=256 packing)
        qvT4z = np.zeros((P, NH, 4, 2, P), F8)
        quTz4 = np.zeros((P, NH, 2, Q), F8)
        for h in range(NH):
            hc = h // 2
            r = hc % 2
            p0 = DK * (h % 2)
            qv = qvT_full[p0:p0 + DK, hc, 512 * c:512 * c + 512]
            # target slice axes (p, qt, m) <- qv reshaped (p, qt, m)
            qvT4z[p0:p0 + DK, h, :, r, :] = qv.reshape(DK, 4, P)
            quTz4[p0:p0 + DK, h, r, :] = quT_full[
                p0:p0 + DK, hc, 512 * c:512 * c + 512]

        kT4_c = np.ascontiguousarray(kT_f8[:, :, tsl])

        vv = vT_f8[:, :, tsl]                                     # (128,6,2048)
        vmat = np.ascontiguousarray(
            vv.transpose(1, 0, 2).reshape(H, T))                  # (768,2048)=v.T
        arr = vmat.reshape(NH, DK, 16, P).transpose(0, 3, 2, 1)   # (12,128,16,64)
        vb_c = np.zeros((P, NH, 8, 2, 96), F8)
        vb_c[:, :, :, :, :DK] = arr.reshape(NH, P, 8, 2, DK).transpose(
            1, 0, 2, 3, 4)
        vb_c[:, :, :, :, DK] = 1.0
        entry = {
            "qvT4z": qvT4z,
            "pTw": np.ascontiguousarray(pT_f8[:, :, w0:w0 + WWIN]),
            "quTz4": quTz4,
            "kT4": kT4_c,
            "vb2": vb_c,
            "xq": np.ascontiguousarray(
                hf[T * b_ + q0: T * b_ + q0 + 512].reshape(4, P, H)
                .transpose(1, 0, 2)),
            "Wo": wo_c, "W1": w1_c, "W2": w2_c, "b1c": b1_c,
        }
        if affine:
            entry["vecs"] = vecs
        if use_mask:
            m = f32(mask[b_])
            mT = m.T[:, q0:q0 + 512]                              # (2048,512) j,q
            entry["maskb"] = (mT.reshape(16, P, 512).transpose(1, 0, 2)
                              * np.float32(-240.0)).astype(F8)
            entry["keepb"] = (1.0 - mT.reshape(16, P, 512)
                              .transpose(1, 0, 2)).astype(F8)
        in2.append(entry)

    PROFILE["in2"] = in2
    _res2 = run_bass_kernel_spmd(d2, in2, core_ids=list(range(NCORE)),
                                 trace=_trace)
    PROFILE["d2_ns"] = _res2.exec_time_ns
    PROFILE["d2_res"] = _res2
    r2 = _res2.results

    outp = np.zeros((B, T, H), np.float32)
    for c in range(NCORE):
        b_ = c // 4
        q0 = 512 * (c % 4)
        outp[b_, q0:q0 + 512] = r2[c]["out"].transpose(1, 0, 2).reshape(512, H)
    return outp


# revision 25
# speedup vs baseline: 1.1848x; 1.0111x over previous
"""Trainium2 Bass kernel for a Transformer-XL style BertLayer (relative attention).

Sharding (8 NeuronCores, full inputs in / full output out):
  Dispatch 1: token-sharded transposed projections qT (bf16) / kT/vT/pT (fp8).
  Host: reassemble; add pos_bias_u/v; build fp8 operands; query-split for
    dispatch 2 (core c: batch c//4, queries [512*(c%4), +512)).
  Dispatch 2: attention with keys-on-partitions. All score/FFN matmuls use
    fp8 DoubleRow (0.5 cyc/col):
      - BD rect: 4-head zero-padded stationary qvT4z against dense pT pairs.
      - content: dense kT pair stationary against per-head zero-padded quTz4
        moving (zero rows of other heads contribute nothing).
      - rel-shift via DRAM rect + sheared fp8->f32 casting SWDGE readback,
        PE-transposed (f32r) into the content-score PSUM accumulation.
      - softmax denominators ride as a ones-column in V (fp8 DR); all 12
        head denominators are reciprocal'd in one batched DVE op at the end.
      - FFN in fp8 DoubleRow (W1/W2/x1T/h1 fp8), FFN2 accumulated fully in
        PSUM per query block with LN2 overlapped per block.
"""

import os
import sys
import numpy as np
import ml_dtypes

sys.path.insert(0, "/opt/trn_rl_repo")

import concourse.bass as bass
import concourse.mybir as mybir
import concourse.tile as tile
from concourse import bacc
from concourse.bass_utils import run_bass_kernel_spmd
from concourse.masks import make_identity

BF = ml_dtypes.bfloat16
F8 = ml_dtypes.float8_e4m3
F32, BF16, F32R = mybir.dt.float32, mybir.dt.bfloat16, mybir.dt.float32r
FP8 = mybir.dt.float8e4
DR = mybir.MatmulPerfMode.DoubleRow
AFT = mybir.ActivationFunctionType
ALU = mybir.AluOpType
AXX = mybir.AxisListType.X

B, T, H, NH, DK = 2, 2048, 768, 12, 64
P = 128
FC = H // P            # 6 feature chunks
GC = 3072 // P         # 24 intermediate chunks
Q = 512                # queries per core
NCORE = 8
WWIN = 2560            # pT window width per core
BDW = 2176             # BD rect row width (2175 used + 1 pad)
LN_EPS = 1e-5
WSC = 16.0             # fp8 weight pre-scale (host side)
CSC = 32.0             # ctxT pre-scale

_cache = {}
PROFILE = {}


def _build_d1():
    nc = bacc.Bacc(None, target_bir_lowering=False)
    xT = nc.dram_tensor("xT", [P, FC, Q], FP8, kind="ExternalInput")
    posT = nc.dram_tensor("posT", [P, FC, Q], FP8, kind="ExternalInput")
    ws = {n: nc.dram_tensor(n, [P, FC, 3, 2, P], FP8, kind="ExternalInput")
          for n in ("Wq", "Wk", "Wv", "Wp")}
    bs = {n: nc.dram_tensor(n, [P, FC], F32, kind="ExternalInput")
          for n in ("bq", "bk", "bv")}
    odt = {"qT": BF16, "kT": FP8, "vT": FP8, "pT": FP8}
    outs = {n: nc.dram_tensor(n, [P, FC, Q], odt[n], kind="ExternalOutput")
            for n in ("qT", "kT", "vT", "pT")}

    with tile.TileContext(nc) as tc:
        with tc.tile_pool(name="sb", bufs=2) as sb, \
             tc.tile_pool(name="wp", bufs=2) as wp, \
             tc.tile_pool(name="ps", bufs=3, space="PSUM") as psp:
            xT_sb = sb.tile([P, FC, Q], FP8, tag="x")
            nc.sync.dma_start(xT_sb[:], xT[:])
            posT_sb = sb.tile([P, FC, Q], FP8, tag="p")
            nc.scalar.dma_start(posT_sb[:], posT[:])
            bias_sb = {}
            for n in bs:
                t = sb.tile([P, FC], F32, tag=n)
                nc.scalar.dma_start(t[:], bs[n][:])
                bias_sb[n] = t

            ev = 0
            for wn, bn, on, src in (("Wq", "bq", "qT", xT_sb),
                                    ("Wk", "bk", "kT", xT_sb),
                                    ("Wv", "bv", "vT", xT_sb),
                                    ("Wp", None, "pT", posT_sb)):
                w_sb = wp.tile([P, FC, 3, 2, P], FP8, tag="w")
                (nc.sync if ev % 2 == 0 else nc.scalar).dma_start(
                    w_sb[:], ws[wn][:])
                ev += 1
                o_sb = sb.tile([P, FC, Q], odt[on], tag="o" + on)
                for dc in range(FC):
                    ps = psp.tile([P, Q], F32, tag="ps")
                    for fcp in range(3):
                        nc.tensor.matmul(ps[:], w_sb[:, dc, fcp],
                                         src[:, 2 * fcp:2 * fcp + 2, :],
                                         perf_mode=DR,
                                         start=(fcp == 0), stop=(fcp == 2))
                    if bn is None:
                        nc.scalar.activation(o_sb[:, dc], ps[:], AFT.Copy,
                                             scale=1.0 / WSC)
                    else:
                        nc.scalar.activation(o_sb[:, dc], ps[:], AFT.Identity,
                                             scale=1.0 / WSC,
                                             bias=bias_sb[bn][:, dc:dc + 1])
                nc.sync.dma_start(outs[on][:], o_sb[:])
    nc.compile()
    return nc


def _build_d2(use_mask: bool, affine: bool):
    nc = bacc.Bacc(None, target_bir_lowering=False)
    qvT4z = nc.dram_tensor("qvT4z", [P, NH, 4, 2, P], FP8, kind="ExternalInput")
    pTw = nc.dram_tensor("pTw", [P, FC, WWIN], FP8, kind="ExternalInput")
    quTz4 = nc.dram_tensor("quTz4", [P, NH, 2, Q], FP8, kind="ExternalInput")
    kT4 = nc.dram_tensor("kT4", [P, FC, T], FP8, kind="ExternalInput")
    vb2 = nc.dram_tensor("vb2", [P, NH, 8, 2, 96], FP8, kind="ExternalInput")
    Wo = nc.dram_tensor("Wo", [P, FC, H], FP8, kind="ExternalInput")
    W1 = nc.dram_tensor("W1", [P, FC, 3072], BF16, kind="ExternalInput")
    W2 = nc.dram_tensor("W2", [P, GC, H], BF16, kind="ExternalInput")
    b1c = nc.dram_tensor("b1c", [P, GC], F32, kind="ExternalInput")
    xq = nc.dram_tensor("xq", [P, 4, H], F32, kind="ExternalInput")
    if affine:
        # rows: 0=bo 1=b2 2=ln1_g 3=ln1_b 4=ln2_g 5=ln2_b
        vecs = nc.dram_tensor("vecs", [P, 6, H], F32, kind="ExternalInput")
    if use_mask:
        maskb = nc.dram_tensor("maskb", [P, 16, Q], FP8, kind="ExternalInput")
        keepb = nc.dram_tensor("keepb", [P, 16, Q], FP8, kind="ExternalInput")
    out = nc.dram_tensor("out", [P, 4, H], F32, kind="ExternalOutput")

    wb = 1 if (use_mask or affine) else 2
    shb = 1 if affine else 2
    with tile.TileContext(nc) as tc:
        with tc.tile_pool(name="res", bufs=1) as res, \
             tc.tile_pool(name="stream", bufs=3) as stream, \
             tc.tile_pool(name="work", bufs=2) as work, \
             tc.tile_pool(name="dram", bufs=4, space="DRAM") as dpool:

            # ---------------- resident loads (attention-critical first) ----
            qvT4z_sb = res.tile([P, NH, 4, 2, P], FP8, tag="qvT4z")
            nc.sync.dma_start(qvT4z_sb[:, 0:4], qvT4z[:, 0:4])
            pT_sb = res.tile([P, FC, WWIN], FP8, tag="pTw")
            nc.sync.dma_start(pT_sb[:, 0:2], pTw[:, 0:2])
            kT4_sb = res.tile([P, FC, T], FP8, tag="kT4")
            nc.sync.dma_start(kT4_sb[:, 0:2], kT4[:, 0:2])
            quTz4_sb = res.tile([P, NH, 2, Q], FP8, tag="quTz4")
            nc.sync.dma_start(quTz4_sb[:, 0:4], quTz4[:, 0:4])
            vb_h0 = stream.tile([P, 8, 2, 96], FP8, tag="vbh", bufs=2)
            nc.sync.dma_start(vb_h0[:], vb2[:, 0])
            nc.sync.dma_start(qvT4z_sb[:, 4:], qvT4z[:, 4:])
            nc.sync.dma_start(pT_sb[:, 2:], pTw[:, 2:])
            nc.sync.dma_start(kT4_sb[:, 2:], kT4[:, 2:])
            nc.sync.dma_start(quTz4_sb[:, 4:], quTz4[:, 4:])
            if use_mask:
                mb_sb = res.tile([P, 16, Q], FP8, tag="maskb")
                nc.scalar.dma_start(mb_sb[:], maskb[:])
                kb_sb = res.tile([P, 16, Q], FP8, tag="keepb")
                nc.scalar.dma_start(kb_sb[:], keepb[:])
            Wo_sb = res.tile([P, FC, H], FP8, tag="Wo")
            nc.scalar.dma_start(Wo_sb[:], Wo[:])
            b1_sb = res.tile([P, GC], F32, tag="b1c")
            nc.scalar.dma_start(b1_sb[:], b1c[:])
            if affine:
                vecs_sb = res.tile([P, 6, H], F32, tag="vecs")
                nc.scalar.dma_start(vecs_sb[:], vecs[:])

            ident_raw = res.tile([P, P], F32, tag="idraw")
            make_identity(nc, ident_raw[:])
            ident = res.tile([P, P], F32R, tag="ident")
            nc.vector.tensor_copy(out=ident[:], in_=ident_raw[:])
            ident_bf = res.tile([P, P], BF16, tag="identbf")
            nc.vector.tensor_copy(out=ident_bf[:], in_=ident_raw[:])
            eps_sb = res.tile([P, 1], F32, tag="eps")
            nc.any.memset(eps_sb[:], LN_EPS)
            ones_b = res.tile([1, DK], BF16, tag="onesb")
            nc.any.memset(ones_b[:], CSC)

            ctxT = res.tile([P, FC, Q], FP8, tag="ctxT")
            ctxTf = res.tile([P, FC, Q], FP8, tag="ctxTf")
            den_b = [res.tile([1, Q], BF16, tag=f"denb{h}",
                              name=f"denb{h}") for h in range(NH)]

            # ---------------- attention ----------------
            with tc.tile_pool(name="ps_bd", bufs=3, space="PSUM") as ps_bd, \
                 tc.tile_pool(name="ps_st", bufs=2, space="PSUM") as ps_st, \
                 tc.tile_pool(name="ps_ctx", bufs=1, space="PSUM") as ps_ctx:
                def stage_A_qt(h, bdd, qt):
                    cg = 2 * (h // 4)
                    loc = 384 - 128 * qt
                    bd_sb = work.tile([P, BDW], FP8, tag="bd_sb", bufs=3)
                    for rc in range(5):
                        w = 512 if rc < 4 else 128
                        ps = ps_bd.tile([P, 512], F32, tag="bd")
                        nc.tensor.matmul(
                            ps[:, :w],
                            qvT4z_sb[:, h, qt],
                            pT_sb[:, cg:cg + 2,
                                  loc + rc * 512: loc + rc * 512 + w],
                            perf_mode=DR, start=True, stop=True)
                        if rc in (1, 4):
                            nc.scalar.activation(
                                bd_sb[:, rc * 512: rc * 512 + w],
                                ps[:, :w], AFT.Copy)
                        else:
                            nc.vector.tensor_copy(
                                out=bd_sb[:, rc * 512: rc * 512 + w],
                                in_=ps[:, :w])
                    nc.sync.dma_start(bdd[qt], bd_sb[:])

                def stage_B(h, bdd, vb_h, bdd_next):
                    hp, hc = DK * (h % 2), h // 2
                    cg = 2 * (h // 4)
                    ctx = ps_ctx.tile([96, Q], F32, tag="ctx")
                    for jq in range(4):
                        if bdd_next is not None:
                            stage_A_qt(h + 1, bdd_next, jq)
                        bdsh = stream.tile([P, 4, 512], F32R, tag="bdsh",
                                           bufs=shb)
                        src = bass.AP(bdd.tensor, bdd.offset + 127 + 512 * jq,
                                      [[BDW - 1, P], [P * BDW, 4], [1, 512]])
                        nc.gpsimd.dma_start(bdsh[:], src)
                        for jph in range(2):
                            jp = 2 * jq + jph
                            st = ps_st.tile([P, 1024], F32, tag="st")
                            for half in range(2):
                                jt = 2 * jp + half
                                co = 512 * half
                                for qt in range(4):
                                    nc.tensor.matmul(
                                        st[:, co + qt * P: co + (qt + 1) * P
                                           ].bitcast(F32R),
                                        bdsh[:, qt,
                                             jph * 256 + half * P:
                                             jph * 256 + (half + 1) * P],
                                        ident[:], is_transpose=True,
                                        start=(qt == 0), stop=False)
                                nc.tensor.matmul(
                                    st[:, co:co + 512],
                                    kT4_sb[:, cg:cg + 2,
                                           jt * P:(jt + 1) * P],
                                    quTz4_sb[:, h],
                                    perf_mode=DR, start=False, stop=True)
                                if use_mask:
                                    nc.vector.tensor_tensor(
                                        st[:, co:co + 512], st[:, co:co + 512],
                                        mb_sb[:, jt, :], ALU.add)
                            e2 = work.tile([P, 2, Q], FP8, tag="e2",
                                           bufs=3)
                            nc.scalar.activation(e2[:], st[:],
                                                 AFT.Exp, scale=0.125)
                            if use_mask:
                                for half in range(2):
                                    jt = 2 * jp + half
                                    nc.vector.tensor_tensor(
                                        e2[:, half], e2[:, half],
                                        kb_sb[:, jt, :], ALU.mult)
                            nc.tensor.matmul(ctx[:], vb_h[:, jp], e2[:],
                                             perf_mode=DR,
                                             start=(jp == 0), stop=(jp == 7))
                    # stash ctx fp8 + den row (frees ctx bank fast),
                    # then reciprocal off SBUF so V(h+1) isn't blocked
                    nc.vector.tensor_copy(out=ctxTf[hp:hp + DK, hc, :],
                                          in_=ctx[:DK, :])
                    den_s = work.tile([1, Q], BF16, tag="den_s", bufs=2)
                    nc.scalar.mul(den_s[:], ctx[DK:DK + 1, :], 1.0)
                    with nc.allow_low_precision(reason="denominator bf16"):
                        nc.vector.reciprocal(den_b[h][:], den_s[:])

                bdd_h = dpool.tile([4, P, BDW], FP8, tag="bdd")
                for qt in range(4):
                    stage_A_qt(0, bdd_h, qt)
                for h in range(NH):
                    if h == 0:
                        vb_h = vb_h0
                    else:
                        vb_h = stream.tile([P, 8, 2, 96], FP8, tag="vbh",
                                           bufs=2)
                        nc.sync.dma_start(vb_h[:], vb2[:, h])
                    if h + 1 < NH:
                        bdd_next = dpool.tile([4, P, BDW], FP8, tag="bdd")
                    else:
                        bdd_next = None
                    stage_B(h, bdd_h, vb_h, bdd_next)
                    bdd_h = bdd_next

            # ---------------- Wo + LN1 + x1T ----------------
            x1f = res.tile([P, 4, H], F32, tag="x1f")
            x1T = res.tile([P, FC, Q], BF16, tag="x1T")

            def layer_norm(dst, r, s1, g_row, b_row, bf_copy=None):
                # r: (P, H) f32 work AP; s1: (P, 1) row-sum of r
                nm = work.tile([P, 1], F32, tag="nm")
                nc.scalar.mul(nm[:], s1, -1.0 / H)
                nc.vector.tensor_scalar(r, r, nm[:], None, op0=ALU.add)
                sq = work.tile([P, H], F32 if affine else BF16,
                               tag="sq", bufs=wb)
                s2 = work.tile([P, 1], F32, tag="s2")
                nc.vector.scalar_tensor_tensor(
                    sq[:], r, 0.0, r, op0=ALU.add, op1=ALU.mult,
                    accum_out=s2[:])
                sd = work.tile([P, 1], F32, tag="sd")
                nc.scalar.activation(sd[:], s2[:], AFT.Sqrt, scale=1.0 / H,
                                     bias=eps_sb[:, 0:1])
                rstd = work.tile([P, 1], F32, tag="rstd")
                nc.vector.reciprocal(rstd[:], sd[:])
                if affine:
                    nc.vector.tensor_scalar(r, r, rstd[:], None, op0=ALU.mult)
                    nc.vector.tensor_tensor(sq[:], r, g_row, ALU.mult)
                    nc.vector.tensor_tensor(dst, sq[:], b_row, ALU.add)
                else:
                    nc.vector.tensor_scalar(dst, r, rstd[:], None,
                                            op0=ALU.mult)
                if bf_copy is not None:
                    nc.vector.tensor_copy(out=bf_copy, in_=dst)

            with tc.tile_pool(name="ps_ao", bufs=2, space="PSUM") as ps_ao, \
                 tc.tile_pool(name="ps_tr", bufs=2, space="PSUM") as ps_tr, \
                 tc.tile_pool(name="ps_bc", bufs=2, space="PSUM") as ps_bc:
                for h in range(NH):
                    hp, hc = DK * (h % 2), h // 2
                    bc = ps_bc.tile([DK, Q], F32, tag="bc")
                    nc.tensor.matmul(bc[:], ones_b[:], den_b[h][:],
                                     start=True, stop=True)
                    nc.vector.tensor_tensor(ctxT[hp:hp + DK, hc, :],
                                            ctxTf[hp:hp + DK, hc, :], bc[:],
                                            ALU.mult)
                for qt in range(4):
                    ao = ps_ao.tile([P, H], F32, tag="ao")
                    for hcp in range(FC // 2):
                        for c0, c1 in ((0, 512), (512, 768)):
                            nc.tensor.matmul(
                                ao[:, c0:c1],
                                ctxT[:, 2 * hcp:2 * hcp + 2,
                                     qt * P:(qt + 1) * P],
                                Wo_sb[:, 2 * hcp:2 * hcp + 2, c0:c1],
                                perf_mode=DR,
                                start=(hcp == 0), stop=(hcp == FC // 2 - 1))
                    xqt = stream.tile([P, H], F32, tag="xqt", bufs=2)
                    nc.scalar.dma_start(xqt[:], xq[:, qt])
                    resid = work.tile([P, H], F32, tag="resid")
                    s1 = work.tile([P, 1], F32, tag="s1")
                    nc.vector.scalar_tensor_tensor(
                        resid[:], ao[:], 1.0 / (WSC * CSC), xqt[:],
                        op0=ALU.mult, op1=ALU.add,
                        accum_out=None if affine else s1[:])
                    if affine:
                        nc.vector.scalar_tensor_tensor(
                            resid[:], resid[:], 1.0, vecs_sb[:, 0],
                            op0=ALU.mult, op1=ALU.add, accum_out=s1[:])
                    x1b = work.tile([P, H], BF16, tag="x1b")
                    layer_norm(x1f[:, qt], resid[:], s1[:],
                               vecs_sb[:, 2] if affine else None,
                               vecs_sb[:, 3] if affine else None,
                               bf_copy=x1b[:])
                    tp = ps_tr.tile([P, H], BF16, tag="tr")
                    for fcc in range(FC):
                        nc.tensor.matmul(tp[:, fcc * P:(fcc + 1) * P],
                                         x1b[:, fcc * P:(fcc + 1) * P],
                                         ident_bf[:], is_transpose=True,
                                         start=(fcc == 0), stop=(fcc == 5))
                    nc.vector.tensor_copy(
                        out=x1T[:, :, qt * P:(qt + 1) * P], in_=tp[:])

            # ---------------- FFN (bf16) ----------------
            h1_tiles = [res.tile([P, Q], BF16, tag=f"h1_{gc}",
                                 name=f"h1_{gc}") for gc in range(GC)]
            o_acc = [res.tile([P, H], BF16, tag=f"oa{qt}",
                              name=f"oa{qt}") for qt in range(4)]
            with tc.tile_pool(name="ps_h1", bufs=2, space="PSUM") as ps_h1, \
                 tc.tile_pool(name="ps_o2", bufs=2, space="PSUM") as ps_o2:
                for cc in range(4):
                    w2c = stream.tile([P, 6, H], BF16, tag="w2c", bufs=2)
                    nc.sync.dma_start(w2c[:], W2[:, cc * 6:(cc + 1) * 6, :])
                    for gi in range(6):
                        gc = cc * 6 + gi
                        w1s = stream.tile([P, FC, P], BF16, tag="w1s",
                                          bufs=3)
                        nc.sync.dma_start(w1s[:],
                                          W1[:, :, gc * P:(gc + 1) * P])
                        hp1 = ps_h1.tile([P, Q], F32, tag="h1")
                        for fc in range(FC):
                            nc.tensor.matmul(hp1[:], w1s[:, fc],
                                             x1T[:, fc, :],
                                             start=(fc == 0),
                                             stop=(fc == FC - 1))
                        nc.scalar.activation(h1_tiles[gc][:], hp1[:],
                                             AFT.Gelu,
                                             bias=b1_sb[:, gc:gc + 1])
                    for qt in range(4):
                        po = ps_o2.tile([P, 1024], F32, tag="po")
                        for gi in range(6):
                            gc = cc * 6 + gi
                            for c0, c1 in ((0, 512), (512, 768)):
                                nc.tensor.matmul(
                                    po[:, c0:c1],
                                    h1_tiles[gc][:, qt * P:(qt + 1) * P],
                                    w2c[:, gi, c0:c1],
                                    start=(gi == 0), stop=(gi == 5))
                        if cc == 0:
                            nc.vector.tensor_copy(out=o_acc[qt][:],
                                                  in_=po[:, :H])
                        else:
                            nc.vector.tensor_tensor(o_acc[qt][:],
                                                    o_acc[qt][:],
                                                    po[:, :H], ALU.add)
                for qt in range(4):
                    r2 = work.tile([P, H], F32, tag="resid2", bufs=wb)
                    s1b = work.tile([P, 1], F32, tag="s1b")
                    nc.vector.scalar_tensor_tensor(
                        r2[:], o_acc[qt][:], 1.0, x1f[:, qt],
                        op0=ALU.mult, op1=ALU.add,
                        accum_out=None if affine else s1b[:])
                    if affine:
                        nc.vector.scalar_tensor_tensor(
                            r2[:], r2[:], 1.0, vecs_sb[:, 1],
                            op0=ALU.mult, op1=ALU.add, accum_out=s1b[:])
                    o_sb = work.tile([P, H], F32, tag="osb")
                    layer_norm(o_sb[:], r2[:], s1b[:],
                               vecs_sb[:, 4] if affine else None,
                               vecs_sb[:, 5] if affine else None)
                    nc.sync.dma_start(out[:, qt], o_sb[:])
    nc.compile()
    return nc


# --------------------------------------------------------------------------
def _chunk_pf(w):
    """(768, X) -> (128, 6, X) with row d' = 128*chunk + partition."""
    return np.ascontiguousarray(w.reshape(FC, P, -1).transpose(1, 0, 2))


def kernel(hidden_states, attention_mask, pos_emb,
           Wq, bq, Wk, bk, Wv, bv, Wp, pos_bias_u, pos_bias_v, Wo, bo,
           ln1_g, ln1_b, W1, b1, W2, b2, ln2_g, ln2_b):
    f32 = lambda x: np.asarray(x, dtype=np.float32)
    hidden_states = f32(hidden_states)
    pos_emb = f32(pos_emb)
    mask = np.asarray(attention_mask)
    use_mask = bool(mask.any())
    affine = not (np.all(f32(ln1_g) == 1) and np.all(f32(ln1_b) == 0)
                  and np.all(f32(ln2_g) == 1) and np.all(f32(ln2_b) == 0)
                  and np.all(f32(bo) == 0) and np.all(f32(b2) == 0))

    if "d1" not in _cache:
        _cache["d1"] = _build_d1()
    key = ("d2", use_mask, affine)
    if key not in _cache:
        _cache[key] = _build_d2(use_mask, affine)
    d1, d2 = _cache["d1"], _cache[key]

    hf = hidden_states.reshape(B * T, H)
    xT_full = _chunk_pf(np.ascontiguousarray(hf.T)).astype(F8)
    posT_pad = np.zeros((H, 4096), np.float32)
    posT_pad[:, :2 * T - 1] = pos_emb[0].T
    posT_full = _chunk_pf(posT_pad).astype(F8)

    def _w_d1(w):
        # [p, dc, fcp, t, m] = WSC * w[256*fcp + 128*t + p, 128*dc + m]
        return np.ascontiguousarray(
            (f32(w) * WSC).reshape(3, 2, P, FC, P).transpose(2, 3, 0, 1, 4)
        ).astype(F8)

    wq_c, wk_c, wv_c, wp_c = map(_w_d1, (Wq, Wk, Wv, Wp))
    bq_c = f32(bq).reshape(FC, P).T.copy()
    bk_c = f32(bk).reshape(FC, P).T.copy()
    bv_c = f32(bv).reshape(FC, P).T.copy()

    in1 = []
    for c in range(NCORE):
        sl = slice(512 * c, 512 * c + 512)
        in1.append({
            "xT": np.ascontiguousarray(xT_full[:, :, sl]),
            "posT": np.ascontiguousarray(posT_full[:, :, sl]),
            "Wq": wq_c, "Wk": wk_c, "Wv": wv_c, "Wp": wp_c,
            "bq": bq_c, "bk": bk_c, "bv": bv_c,
        })
    _trace = bool(os.environ.get("BERT_KERNEL_TRACE"))
    _res1 = run_bass_kernel_spmd(d1, in1, core_ids=list(range(NCORE)),
                                 trace=_trace)
    PROFILE["d1_ns"] = _res1.exec_time_ns
    PROFILE["d1_res"] = _res1
    r1 = _res1.results

    qT_full = np.concatenate([r["qT"] for r in r1], axis=2).astype(np.float32)
    kT_f8 = np.concatenate([r["kT"] for r in r1], axis=2)
    vT_f8 = np.concatenate([r["vT"] for r in r1], axis=2)
    pT_f8 = np.concatenate([r["pT"] for r in r1], axis=2)
    pT_f8[:, :, 2 * T - 1:] = 0

    pbu_c = f32(pos_bias_u).reshape(NH * DK).reshape(FC, P).T.copy()
    pbv_c = f32(pos_bias_v).reshape(NH * DK).reshape(FC, P).T.copy()
    quT_full = (qT_full + pbu_c[:, :, None]).astype(F8)
    qvT_full = (qT_full + pbv_c[:, :, None]).astype(F8)

    wo_c = (_chunk_pf(f32(Wo)) * WSC).astype(F8)
    w1_c = _chunk_pf(f32(W1)).astype(BF)
    w2_c = np.ascontiguousarray(
        f32(W2).reshape(GC, P, H).transpose(1, 0, 2)).astype(BF)
    b1_c = f32(b1).reshape(GC, P).T.copy()
    if affine:
        vecs = np.stack([np.broadcast_to(f32(x), (P, H)) for x in
                         (bo, b2, ln1_g, ln1_b, ln2_g, ln2_b)], axis=1).copy()

    in2 = []
    for c in range(NCORE):
        b_ = c // 4
        q0 = 512 * (c % 4)
        w0 = 1536 - q0
        tsl = slice(T * b_, T * b_ + T)

        # zero-padded DoubleRow stationaries/movings (4-head K=256 packing)
        qvT4z = np.zeros((P, NH, 4, 2, P), F8)
        quTz4 = np.zeros((P, NH, 2, Q), F8)
        for h in range(NH):
            hc = h // 2
            r = hc % 2
            p0 = DK * (h % 2)
            qv = qvT_full[p0:p0 + DK, hc, 512 * c:512 * c + 512]
            # target slice axes (p, qt, m) <- qv reshaped (p, qt, m)
            qvT4z[p0:p0 + DK, h, :, r, :] = qv.reshape(DK, 4, P)
            quTz4[p0:p0 + DK, h, r, :] = quT_full[
                p0:p0 + DK, hc, 512 * c:512 * c + 512]

        kT4_c = np.ascontiguousarray(kT_f8[:, :, tsl])

        vv = vT_f8[:, :, tsl]                                     # (128,6,2048)
        vmat = np.ascontiguousarray(
            vv.transpose(1, 0, 2).reshape(H, T))                  # (768,2048)=v.T
        arr = vmat.reshape(NH, DK, 16, P).transpose(0, 3, 2, 1)   # (12,128,16,64)
        vb_c = np.zeros((P, NH, 8, 2, 96), F8)
        vb_c[:, :, :, :, :DK] = arr.reshape(NH, P, 8, 2, DK).transpose(
            1, 0, 2, 3, 4)
        vb_c[:, :, :, :, DK] = 1.0
        entry = {
            "qvT4z": qvT4z,
            "pTw": np.ascontiguousarray(pT_f8[:, :, w0:w0 + WWIN]),
            "quTz4": quTz4,
            "kT4": kT4_c,
            "vb2": vb_c,
            "xq": np.ascontiguousarray(
                hf[T * b_ + q0: T * b_ + q0 + 512].reshape(4, P, H)
                .transpose(1, 0, 2)),
            "Wo": wo_c, "W1": w1_c, "W2": w2_c, "b1c": b1_c,
        }
        if affine:
            entry["vecs"] = vecs
        if use_mask:
            m = f32(mask[b_])
            mT = m.T[:, q0:q0 + 512]                              # (2048,512) j,q
            entry["maskb"] = (mT.reshape(16, P, 512).transpose(1, 0, 2)
                              * np.float32(-240.0)).astype(F8)
            entry["keepb"] = (1.0 - mT.reshape(16, P, 512)
                              .transpose(1, 0, 2)).astype(F8)
        in2.append(entry)

    PROFILE["in2"] = in2
    _res2 = run_bass_kernel_spmd(d2, in2, core_ids=list(range(NCORE)),
                                 trace=_trace)
    PROFILE["d2_ns"] = _res2.exec_time_ns
    PROFILE["d2_res"] = _res2
    r2 = _res2.results

    outp = np.zeros((B, T, H), np.float32)
    for c in range(NCORE):
        b_ = c // 4
        q0 = 512 * (c % 4)
        outp[b_, q0:q0 + 512] = r2[c]["out"].transpose(1, 0, 2).reshape(512, H)
    return outp
